# revision 1
# baseline (speedup 1.0000x reference)
"""Distributed Trainium2 Bass kernel for a 16-head causal RoPE attention layer.

Problem: B=2, T=2048, D=1024, H=16, HS=64 (fp32 reference).

Sharding (8 cores): core = b*4 + g, b in {0,1} (batch), g in {0..3} (group of
4 heads).  Each core computes Q/K/V projections for its 256 head-dims, runs
causal flash-style attention for its 4 heads, and applies its 256-row slice
of Wo, producing a partial [T, D] output.  The host sums the 4 partials per
batch and adds bo.  No on-device collectives.

Pipeline (v2): the softmax exp on ScalarE is the throughput floor
(~55us/core of pure data at 1 elem/lane/cycle), so the whole kernel is
arranged to keep ScalarE maximally busy on exp and everything else
overlapped under it:
  - scores for a HEAD PAIR share one [128, 2x512] PSUM tile (head A cols
    0:512, head B 512:1024) written by two row-group-concurrent 64-row
    matmuls (tile_position (0,0)/(64,0) auto-derived), so each exp op
    covers 1024 cols -> half the ACT per-op pipeline overhead.
  - attention is q-quarter major (512 q cols per step) so Y for both
    heads fits one [128, 2x512] PSUM tile; flash accumulation over k
    tiles; denominators via 64 ones-columns in the V stationary.
  - q/k biases are folded into the projection matmuls as a 9th
    contraction row (ones moving row), evictions are pure DVE casts.
  - RoPE runs in 512-col chunks (cast -> 4 DVE partition-shift copies ->
    2 DVE muls -> GpSimd add) so attention starts early.
  - normalize: 1/r = exp(-ln r) on ScalarE over both heads at once.
  - diagonal 128x128 blocks: exp trimmed to the valid cols, triangular
    mask applied multiplicatively on DVE.
PSUM: scores 2 bufs x 2 banks + Y 1 buf x 2 banks + proj/outproj ring
2 bufs x 1 bank = 8 banks exactly.
"""

import numpy as np
import ml_dtypes

import concourse.bass as bass
import concourse.mybir as mybir
import concourse.tile as tile
from concourse.bass_utils import run_bass_kernel_spmd

BF16 = mybir.dt.bfloat16
F32 = mybir.dt.float32

B, T, D = 2, 2048, 1024
H, HS = 16, 64
THETA = 10000.0
NCORES = 8
HG = 4            # heads per core
HD = HG * HS      # head dims per core = 256
SCALE = 1.0 / 8.0  # 1/sqrt(HS)
NEG = -1.0e5       # additive mask for padded keys (exp underflows to 0)

_NC = None


_SELF_SEM = {
    "EngineType.Activation": "Activation_",
    "EngineType.DVE": "DVE_",
    "EngineType.PE": "PE_",
    "EngineType.Pool": "Pool_",
}


def _split_multi_waits(nc):
    """walrus codegen accepts at most ONE semaphore wait per engine
    instruction (the 64B ISA structs have a single EVENTS slot); Tile's
    scheduler freely emits several.  Hoist all but the last wait of each
    instruction onto inserted same-engine EventSemaphore (poll_sem) ops,
    which preserves semantics exactly (engines execute sequentially).

    Additionally drop ge-waits on the instruction's OWN engine semaphore
    for compute engines: those guard WAW/WAR against earlier instructions
    of the same in-order engine, which program order already guarantees
    (each op's writes drain before the next op's visible effects).  Tile
    emits one before nearly every exp in the attention loop; at ~100ns of
    sequencer dispatch each they are pure overhead."""
    def _names(args):
        out = set()
        for a in args:
            for attr in ("memref", "name"):
                v = getattr(a, attr, None)
                if isinstance(v, str):
                    out.add(v.removesuffix("_set"))
            t = getattr(a, "tensor", None)
            if t is not None and isinstance(getattr(t, "name", None), str):
                out.add(t.name)
        return out

    # per-engine written/read tensor sets: an op READING an own-engine-
    # written tensor (RAW) or WRITING an own-engine-read tensor (WAR) has a
    # genuine same-engine hazard through the deep pipeline, so its self-wait
    # must survive; pure WAW through the in-order write port is safe.
    eng_written = {}
    eng_read = {}
    _COMPUTE = {"InstActivation", "InstTensorTensor", "InstTensorCopy",
                "InstMatmult", "InstLdweights", "InstMemset",
                "InstTensorScalarPtr", "InstTensorReduce"}
    for f in nc.m.functions:
        for blk in f.blocks:
            for inst in blk.instructions:
                if type(inst).__name__ in _COMPUTE:
                    e = str(inst.engine)
                    eng_written.setdefault(e, set()).update(_names(inst.outs))
                    eng_read.setdefault(e, set()).update(_names(inst.ins))

    n = 0
    for f in nc.m.functions:
        for blk in f.blocks:
            il = blk.instructions
            i = 0
            while i < len(il):
                inst = il[i]
                si = inst.sync_info
                if si is None or not si.on_wait:
                    i += 1
                    continue
                waits = list(si.on_wait)
                eng = str(inst.engine)
                selfpfx = _SELF_SEM.get(eng)
                if (selfpfx is not None
                        and type(inst).__name__ in (
                            "InstActivation", "InstMatmult", "InstLdweights",
                            "InstTensorTensor", "InstTensorCopy", "InstMemset")
                        and not (_names(inst.ins) & eng_written.get(eng, set()))
                        and not (_names(inst.outs) & eng_read.get(eng, set()))):
                    kept = [w for w in waits
                            if not (w.wait_mode == "sem-ge-imm"
                                    and w.ant_name.startswith(selfpfx))]
                    if len(kept) != len(waits):
                        waits = kept
                        inst.sync_info = mybir.SyncInfo(
                            on_wait=waits, on_update=list(si.on_update))
                if len(waits) > 1:
                    for w in waits[:-1]:
                        es = mybir.InstEventSemaphore(name=f"I-wsplit-{n}")
                        n += 1
                        es.engine = inst.engine
                        es.sync_info = mybir.SyncInfo(on_wait=[w], on_update=[])
                        nc.register_instruction(es)
                        il.insert(i, es)
                        i += 1
                    inst.sync_info = mybir.SyncInfo(
                        on_wait=[waits[-1]], on_update=list(si.on_update))
                i += 1
    return n


def _dedup_ldweights(nc):
    """bass emits one InstLdweights per InstMatmult.  When a later
    InstLdweights loads the IDENTICAL weights AP that is already resident
    in the PE array (no other InstLdweights in between), the reload is
    redundant: MATMUL does not self-load for 16-bit dtypes.  Delete it,
    folding its waits into the following matmult (whose multi-waits are
    then legalized by _split_multi_waits)."""
    def fp(inst):
        a = inst.ins[0]
        return (a.memref, a.offset, str(a.ap), str(a.dtype))

    n = 0
    for f in nc.m.functions:
        for blk in f.blocks:
            il = blk.instructions
            last = None
            i = 0
            while i < len(il):
                inst = il[i]
                tn = type(inst).__name__
                if tn == "InstLdweights":
                    cur = fp(inst)
                    si = inst.sync_info
                    if cur == last and (si is None or not si.on_update):
                        waits = list(si.on_wait) if si is not None else []
                        if waits:
                            j = i + 1
                            while (j < len(il)
                                   and type(il[j]).__name__ != "InstMatmult"):
                                j += 1
                            if j == len(il):
                                i += 1
                                continue
                            mm = il[j]
                            msi = mm.sync_info
                            mw = list(msi.on_wait) if msi is not None else []
                            mu = list(msi.on_update) if msi is not None else []
                            mm.sync_info = mybir.SyncInfo(
                                on_wait=waits + mw, on_update=mu)
                        del il[i]
                        n += 1
                        continue
                    last = cur
                i += 1
    return n


def build_nc():
    nc = bass.Bass()

    xT = nc.declare_dram_parameter("xT", [D, T], BF16, isOutput=False)
    wq = nc.declare_dram_parameter("wq", [D, HD], BF16, isOutput=False)
    wk = nc.declare_dram_parameter("wk", [D, HD], BF16, isOutput=False)
    wv = nc.declare_dram_parameter("wv", [D, HD], BF16, isOutput=False)
    wo = nc.declare_dram_parameter("wo", [HD, D], BF16, isOutput=False)
    # [bq(256) | bk(256)] as a single stationary bias row
    brow = nc.declare_dram_parameter("brow", [1, 2 * HD], BF16, isOutput=False)
    bv = nc.declare_dram_parameter("bv", [1, HD], F32, isOutput=False)
    cos2 = nc.declare_dram_parameter("cos2", [128, T], BF16, isOutput=False)
    sin2 = nc.declare_dram_parameter("sin2", [128, T], BF16, isOutput=False)
    tri2 = nc.declare_dram_parameter("tri2", [128, 256], BF16, isOutput=False)
    kb = nc.declare_dram_parameter("kb", [T], F32, isOutput=False)
    out = nc.declare_dram_parameter("out", [T, D], BF16, isOutput=True)

    NT = T // 512   # 4 T-ranges for projections
    NK = T // 128   # 16 key tiles

    with tile.TileContext(nc) as tc:
        with (
            tc.tile_pool(name="const", bufs=1) as cpool,
            tc.tile_pool(name="xw", bufs=1) as xwpool,
            tc.tile_pool(name="qk", bufs=1) as qkpool,
            tc.tile_pool(name="raw", bufs=3) as rawpool,
            tc.tile_pool(name="p", bufs=6) as ppool,
            tc.tile_pool(name="rec", bufs=2) as rpool,
            tc.tile_pool(name="ev", bufs=3) as evpool,
            tc.tile_pool(name="psP", bufs=2, space="PSUM") as psP,
            tc.tile_pool(name="psS", bufs=2, space="PSUM") as psS,
            tc.tile_pool(name="psY", bufs=1, space="PSUM") as psY,
        ):
            # ---- constant / weight loads ----
            # order matters: wq + the first xT column-chunks gate the first
            # matmul groups, so issue them first; xT is loaded in 512-col
            # chunks so projection groups start after ~1MB, not 4MB.
            wq_sb = xwpool.tile([128, 8, HD], BF16, tag="wq")
            wk_sb = xwpool.tile([128, 8, HD], BF16, tag="wk")
            wv_sb = xwpool.tile([128, 8, HD], BF16, tag="wv")
            wo_sb = xwpool.tile([128, 2, D], BF16, tag="wo")
            brow_sb = cpool.tile([1, 2 * HD], BF16, tag="brow")
            ones_sb = cpool.tile([1, 512], BF16, tag="ones")
            bv_sb = cpool.tile([128, HD], F32, tag="bv")
            wq_r = wq.ap().rearrange("(c p) n -> p c n", p=128)
            wk_r = wk.ap().rearrange("(c p) n -> p c n", p=128)
            # pair-0 (c2=0) weight halves first: the first attention quarter
            # needs only these
            nc.sync.dma_start(wq_sb[:, :, 0:128], wq_r[:, :, 0:128])
            nc.sync.dma_start(brow_sb[:], brow.ap())
            nc.vector.memset(ones_sb[:], 1.0)

            xts = []
            for dc in range(8):
                xt = xwpool.tile([128, T], BF16, tag=f"xt{dc}", name=f"xt{dc}")
                xts.append(xt)

            def load_xt_tr(tr):
                for dc in range(8):
                    nc.sync.dma_start(
                        xts[dc][:, tr * 512:(tr + 1) * 512],
                        xT[dc * 128:(dc + 1) * 128, tr * 512:(tr + 1) * 512],
                    )

            cos_sb = cpool.tile([128, T], BF16, tag="cos")
            sin_sb = cpool.tile([128, T], BF16, tag="sin")
            tri2_sb = cpool.tile([128, 2, 128], BF16, tag="tri2")
            kb_sb = cpool.tile([128, NK], F32, tag="kb")
            # everything the first attention quarter needs (q/k tr0 chunks,
            # rope tables, wv, exp consts) comes before the bulk of xT so
            # the lead-in isn't DMA-gated
            load_xt_tr(0)
            nc.sync.dma_start(wk_sb[:, :, 0:128], wk_r[:, :, 0:128])
            nc.sync.dma_start(cos_sb[:, 0:512], cos2[:, 0:512])
            nc.sync.dma_start(sin_sb[:, 0:512], sin2[:, 0:512])
            nc.sync.dma_start(wv_sb[:], wv.ap().rearrange("(c p) n -> p c n", p=128))
            nc.sync.dma_start(
                tri2_sb[:], tri2.ap().rearrange("p (two q) -> p two q", two=2))
            nc.sync.dma_start(kb_sb[:], kb.ap().rearrange("(t p) -> p t", p=128))
            nc.sync.dma_start(bv_sb[:], bv.ap().to_broadcast((128, HD)))
            load_xt_tr(1)
            nc.sync.dma_start(cos_sb[:, 512:T], cos2[:, 512:T])
            nc.sync.dma_start(sin_sb[:, 512:T], sin2[:, 512:T])
            load_xt_tr(2)
            load_xt_tr(3)
            nc.sync.dma_start(wq_sb[:, :, 128:256], wq_r[:, :, 128:256])
            nc.sync.dma_start(wk_sb[:, :, 128:256], wk_r[:, :, 128:256])
            nc.sync.dma_start(wo_sb[:], wo.ap().rearrange("(c p) n -> p c n", p=128))

            # persistent [128, T] tiles: 2 heads each (rows 0:64 / 64:128)
            qT = [qkpool.tile([128, T], BF16, tag=f"qT{c}", name=f"qT{c}") for c in range(2)]
            kT = [qkpool.tile([128, T], BF16, tag=f"kT{c}", name=f"kT{c}") for c in range(2)]
            yT = [qkpool.tile([128, T], BF16, tag=f"yT{c}", name=f"yT{c}") for c in range(2)]

            # ---- Q^T / K^T projection + RoPE, one 512-col chunk ----
            # ti: 0 = q, 1 = k (selects bias row slice)
            def proj_qk_chunk(ti, wsb, c2, fin, tr):
                lo, hi = tr * 512, (tr + 1) * 512
                ps = psP.tile([128, 512], F32, tag="pp")
                for dc in range(8):
                    nc.tensor.matmul(
                        ps[:],
                        wsb[:, dc, c2 * 128:(c2 + 1) * 128],
                        xts[dc][:, lo:hi],
                        start=(dc == 0),
                        stop=False,
                    )
                # bias via rank-1 update: ones row x bias row
                nc.tensor.matmul(
                    ps[:],
                    brow_sb[0:1, ti * HD + c2 * 128: ti * HD + (c2 + 1) * 128],
                    ones_sb[0:1, :],
                    start=False, stop=True,
                )
                raw = rawpool.tile([128, 512], BF16, tag="raw")
                nc.vector.tensor_copy(raw[:], ps[:])
                # RoPE: fin = raw*cos + rot(raw)*sin_signed
                f = fin
                for (do, di) in ((0, 32), (32, 0), (64, 96), (96, 64)):
                    nc.vector.tensor_copy(f[do:do + 32, lo:hi], raw[di:di + 32, :])
                nc.vector.tensor_mul(f[:, lo:hi], f[:, lo:hi], sin_sb[:, lo:hi])
                nc.vector.tensor_mul(raw[:], raw[:], cos_sb[:, lo:hi])
                # final add on GpSimd (idle) to unload DVE
                nc.gpsimd.tensor_add(f[:, lo:hi], f[:, lo:hi], raw[:])

            # same projection for TWO 512-col chunks, dc-major over two psum
            # tiles: each stationary slice serves both chunks back-to-back so
            # _dedup_ldweights deletes every second (identical) weight load.
            def proj_qk_pair(ti, wsb, c2, fin, trp):
                trs = (2 * trp, 2 * trp + 1)
                pss = [psP.tile([128, 512], F32, tag="pp", name=f"pp{t}")
                       for t in range(2)]
                for dc in range(8):
                    for t in range(2):
                        lo = trs[t] * 512
                        nc.tensor.matmul(
                            pss[t][:],
                            wsb[:, dc, c2 * 128:(c2 + 1) * 128],
                            xts[dc][:, lo:lo + 512],
                            start=(dc == 0),
                            stop=False,
                            skip_group_check=True,
                        )
                for t in range(2):
                    nc.tensor.matmul(
                        pss[t][:],
                        brow_sb[0:1, ti * HD + c2 * 128: ti * HD + (c2 + 1) * 128],
                        ones_sb[0:1, :],
                        start=False, stop=True,
                        skip_group_check=True,
                    )
                for t in range(2):
                    lo = trs[t] * 512
                    hi = lo + 512
                    raw = rawpool.tile([128, 512], BF16, tag="raw")
                    nc.vector.tensor_copy(raw[:], pss[t][:])
                    f = fin
                    for (do, di) in ((0, 32), (32, 0), (64, 96), (96, 64)):
                        nc.vector.tensor_copy(f[do:do + 32, lo:hi], raw[di:di + 32, :])
                    nc.vector.tensor_mul(f[:, lo:hi], f[:, lo:hi], sin_sb[:, lo:hi])
                    nc.vector.tensor_mul(raw[:], raw[:], cos_sb[:, lo:hi])
                    nc.gpsimd.tensor_add(f[:, lo:hi], f[:, lo:hi], raw[:])

            # ---- V projection (normal layout, with bias and ones blocks) ----
            vts = [None] * NK

            def proj_v(kt):
                ps = psP.tile([128, HD], F32, tag="pp")
                for dc in range(8):
                    nc.tensor.matmul(
                        ps[:],
                        xts[dc][:, kt * 128:(kt + 1) * 128],
                        wv_sb[:, dc, :],
                        start=(dc == 0),
                        stop=(dc == 7),
                    )
                vt = xwpool.tile([128, HG, 128], BF16, tag=f"v{kt}", name=f"v{kt}")
                nc.vector.tensor_add(
                    vt[:, :, 0:64],
                    ps[:].rearrange("p (h d) -> p h d", h=HG),
                    bv_sb[:].rearrange("p (h d) -> p h d", h=HG),
                )
                nc.vector.memset(vt[:, :, 64:128], 1.0)
                vts[kt] = vt

            # ---- attention for head pair c2, one q-quarter (512 cols) ----
            # scores/probs/Y for both heads live side by side in one
            # [128, 2, 512] tile: [:, 0, :] = head 2*c2, [:, 1, :] = 2*c2+1.
            def attn_quarter(c2, qq, fillers=()):
                # fillers: closures emitting independent work (V tiles,
                # projections, outproj groups), interspersed between units so
                # the static per-engine schedule never parks a solid filler
                # block between this quarter's exp stream and the next's
                fillers = list(fillers)
                qlo = qq * 512
                last = 4 * qq + 3
                y = psY.tile([128, 2, 512], F32, tag="y", name=f"y{c2}_{qq}")
                for kt in range(last + 1):
                    if fillers and kt % 2 == 1:
                        fillers.pop(0)()
                    j = kt - 4 * qq
                    c = j * 128 if j >= 0 else 0   # first valid col (diag trim)
                    ksl = slice(kt * 128, (kt + 1) * 128)
                    qsl = slice(qlo + c, qlo + 512)
                    s = psS.tile([128, 2, 512], F32, tag="s")
                    # two row-group-concurrent 64-row score matmuls
                    nc.tensor.matmul(
                        s[:, 0, c:], kT[c2][0:64, ksl], qT[c2][0:64, qsl],
                        start=True, stop=True,
                    )
                    nc.tensor.matmul(
                        s[:, 1, c:], kT[c2][64:128, ksl], qT[c2][64:128, qsl],
                        start=True, stop=True,
                    )
                    p = ppool.tile([128, 2, 512], BF16, tag="p")
                    nc.scalar.activation(
                        p[:, :, c:], s[:, :, c:],
                        mybir.ActivationFunctionType.Exp,
                        bias=kb_sb[:, kt:kt + 1], scale=SCALE,
                    )
                    if j >= 0:
                        # diagonal 128x128 blocks of both heads: tri mask
                        nc.vector.tensor_mul(
                            p[:, :, c:c + 128], p[:, :, c:c + 128], tri2_sb[:]
                        )
                    for h in (0, 1):
                        nc.tensor.matmul(
                            y[:, h, c:],
                            vts[kt][:, 2 * c2 + h, :],
                            p[:, h, c:],
                            start=(kt == 0),
                            stop=(kt == last),
                            skip_group_check=True,
                        )
                # normalize both heads at once: 1/r = exp(-ln r)
                lnr = rpool.tile([64, 2, 512], F32, tag="lnr")
                rec = rpool.tile([64, 2, 512], F32, tag="rec")
                nc.scalar.activation(
                    lnr[:], y[64:128, :, :], mybir.ActivationFunctionType.Ln)
                nc.scalar.activation(
                    rec[:], lnr[:], mybir.ActivationFunctionType.Exp,
                    scale=-1.0)
                nc.vector.tensor_mul(
                    yT[c2][0:64, qlo:qlo + 512], y[0:64, 0, :], rec[:, 0, :])
                nc.vector.tensor_mul(
                    yT[c2][64:128, qlo:qlo + 512], y[0:64, 1, :], rec[:, 1, :])
                for fill in fillers:
                    fill()

            def outproj_quarter(qq):
                # partial out for T-tiles of this quarter; host adds bo+reduces
                # c2-outer / dr-inner: each yT stationary slice serves both wo
                # column halves (second load deleted by _dedup_ldweights).
                # Last two quarters evict on ScalarE: the exp stream is done
                # by then, and it shortens the DVE-bound tail.
                for tt in range(4 * qq, 4 * qq + 4):
                    outproj_tt(tt, qq >= 2)

            def outproj_tt(tt, on_scalar):
                    pss = [psP.tile([128, 512], F32, tag="pp", name=f"po{t}")
                           for t in range(2)]
                    for c2 in range(2):
                        for dr in range(2):
                            nc.tensor.matmul(
                                pss[dr][:],
                                yT[c2][:, tt * 128:(tt + 1) * 128],
                                wo_sb[:, c2, dr * 512:(dr + 1) * 512],
                                start=(c2 == 0),
                                stop=(c2 == 1),
                                skip_group_check=True,
                            )
                    for dr in range(2):
                        ev = evpool.tile([128, 512], BF16, tag="ev")
                        if on_scalar:
                            nc.scalar.activation(
                                ev[:], pss[dr][:],
                                mybir.ActivationFunctionType.Identity)
                        else:
                            nc.vector.tensor_copy(ev[:], pss[dr][:])
                        nc.sync.dma_start(
                            out[tt * 128:(tt + 1) * 128, dr * 512:(dr + 1) * 512],
                            ev[:],
                        )

            # ---- emission order == scheduler priority ----
            # quarter qq of pair 0 needs exactly q/k chunks tr<=qq and
            # V tiles kt<=4qq+3, so interleave per quarter: attention
            # starts right after the first 1.5MB of DMA instead of after
            # the whole projection phase.
            # quarters 0/1 use single chunks (tr0 must not wait on tr1's xT
            # DMA in the lead-in); later projections go in reuse-pairs
            for qq in range(2):
                proj_qk_chunk(0, wq_sb, 0, qT[0], qq)
                proj_qk_chunk(1, wk_sb, 0, kT[0], qq)
                for kt in range(4 * qq, 4 * qq + 4):
                    proj_v(kt)
                attn_quarter(0, qq)
            proj_qk_pair(0, wq_sb, 0, qT[0], 1)
            proj_qk_pair(1, wk_sb, 0, kT[0], 1)
            for kt in range(8, 12):
                proj_v(kt)
            attn_quarter(0, 2)
            # V 12-15 before the pair-1 projections: attn(0,3) needs them
            # and the psP ring serves groups in emission order
            for kt in range(12, 16):
                proj_v(kt)
            attn_quarter(0, 3)
            # pair-1 tr0/tr1 projections run during attn(0,3) + transition;
            # tr2/tr3 live in the pair-1 window (which has PE slack) so
            # they stop crowding the pair-0 crunch region.  outproj lands
            # one quarter late so its matmuls fill exp-wait gaps instead of
            # outranking the next quarter's score matmuls.
            proj_qk_pair(0, wq_sb, 1, qT[1], 0)
            proj_qk_pair(1, wk_sb, 1, kT[1], 0)
            attn_quarter(1, 0)
            proj_qk_pair(0, wq_sb, 1, qT[1], 1)
            attn_quarter(1, 1)
            proj_qk_pair(1, wk_sb, 1, kT[1], 1)
            outproj_quarter(0)
            attn_quarter(1, 2)
            outproj_quarter(1)
            attn_quarter(1, 3)
            outproj_quarter(2)
            outproj_quarter(3)
    nd = _dedup_ldweights(nc)
    _split_multi_waits(nc)
    assert nd > 0, f"expected ldweights dedup to fire, got {nd}"
    return nc


def _rope_tables():
    inv_freq = 1.0 / (THETA ** (np.arange(0, HS, 2, dtype=np.float64) / HS))  # [32]
    t = np.arange(T, dtype=np.float64)
    fr = t[:, None] * inv_freq[None, :]          # [T, 32]
    emb = np.concatenate([fr, fr], axis=1)       # [T, 64]
    cos = np.cos(emb).T.astype(np.float32)       # [64, T]
    sin = np.sin(emb).T.astype(np.float32)       # [64, T]
    sin_signed = sin.copy()
    sin_signed[0:32] = -sin_signed[0:32]
    cos2 = np.concatenate([cos, cos], axis=0)            # [128, T]
    sin2 = np.concatenate([sin_signed, sin_signed], 0)   # [128, T]
    return cos2.astype(ml_dtypes.bfloat16), sin2.astype(ml_dtypes.bfloat16)


def _in_maps(x, attention_mask, Wq, bqv, Wk, bkv, Wv, bvv, Wo):
    cos2, sin2 = _rope_tables()
    tri = np.triu(np.ones((128, 128), np.float32))
    tri2 = np.concatenate([tri, tri], axis=1).astype(ml_dtypes.bfloat16)
    bf = ml_dtypes.bfloat16
    xTs = [np.ascontiguousarray(x[b].T).astype(bf) for b in range(B)]
    kbs = [
        np.where(attention_mask[b] != 0, 0.0, NEG).astype(np.float32)
        for b in range(B)
    ]
    maps = []
    for core in range(NCORES):
        b, g = core // 4, core % 4
        sl = slice(g * HD, (g + 1) * HD)
        brow = np.concatenate([bqv[sl], bkv[sl]]).reshape(1, 2 * HD)
        maps.append({
            "xT": xTs[b],
            "wq": np.ascontiguousarray(Wq[:, sl]).astype(bf),
            "wk": np.ascontiguousarray(Wk[:, sl]).astype(bf),
            "wv": np.ascontiguousarray(Wv[:, sl]).astype(bf),
            "wo": np.ascontiguousarray(Wo[sl, :]).astype(bf),
            "brow": brow.astype(bf),
            "bv": bvv[sl].astype(np.float32).reshape(1, HD),
            "cos2": cos2,
            "sin2": sin2,
            "tri2": tri2,
            "kb": kbs[b],
        })
    return maps


def _run(inputs, trace=False):
    global _NC
    if _NC is None:
        _NC = build_nc()
    maps = _in_maps(
        np.asarray(inputs["x"]), np.asarray(inputs["attention_mask"]),
        np.asarray(inputs["Wq"]), np.asarray(inputs["bq"]),
        np.asarray(inputs["Wk"]), np.asarray(inputs["bk"]),
        np.asarray(inputs["Wv"]), np.asarray(inputs["bv"]),
        np.asarray(inputs["Wo"]),
    )
    res = run_bass_kernel_spmd(_NC, maps, core_ids=list(range(NCORES)), trace=trace)
    bo = np.asarray(inputs["bo"], np.float32)
    outs = []
    for b in range(B):
        acc = np.zeros((T, D), np.float32)
        for g in range(4):
            acc += np.asarray(res.results[b * 4 + g]["out"], np.float32)
        outs.append(acc + bo[None, :])
    return np.stack(outs, axis=0), res


def kernel(**inputs):
    out, _ = _run(inputs, trace=False)
    return out



# revision 4
# speedup vs baseline: 1.0053x; 1.0053x over previous
"""Distributed Trainium2 Bass kernel for a 16-head causal RoPE attention layer.

Problem: B=2, T=2048, D=1024, H=16, HS=64 (fp32 reference).

Sharding (8 cores): core = b*4 + g, b in {0,1} (batch), g in {0..3} (group of
4 heads).  Each core computes Q/K/V projections for its 256 head-dims, runs
causal attention for its 4 heads, and applies its 256-row slice of Wo,
producing a partial [T, D] output.  The host sums the 4 partials per batch
and adds bo.  No on-device collectives.

v4: mixed precision split by error sensitivity.  The SCORE path tolerates
fp8 (softmax damps score noise ~10x), the VALUE path does not (v/p/y/wo
errors pass straight through; fp8 there measured 5.3e-2 rel err):
  - Q/K projections: fp8 DoubleRow matmuls (x and Wq/Wk cast to fp8 on the
    host, weights scaled by 32 to clear the fp8 subnormal range; the 1024x
    score scale is folded into the exp scale).  4 DR matmuls per 512-col
    chunk + bf16 rank-1 bias.
  - V projection / attention Y / output projection: bf16 exactly as the
    score-insensitive baseline, with V and out biases as rank-1 matmul
    updates (frees DVE adds).
  - causal masking: the diagonal 128x128 strict upper triangle is added as
    a -1e5 stationary matmul into the scores psum (PE), so exp underflows
    to 0 and DVE never touches the exp->Y path (was 40 tri-mask muls).
ACT is the bottleneck (~81us exp stream + ~21us ln/exp normalize); PE
(~87us), DVE (~77us), Pool (~23us rope adds) hide under it.
PSUM: scores 2x2 banks + Y 2 banks + proj/outproj ring 2x1 = 8 exactly.
"""

import numpy as np
import ml_dtypes

import concourse.bass as bass
import concourse.mybir as mybir
import concourse.tile as tile
from concourse.bass_utils import run_bass_kernel_spmd

BF16 = mybir.dt.bfloat16
F32 = mybir.dt.float32
FP8 = mybir.dt.float8e4
DR = mybir.MatmulPerfMode.DoubleRow

B, T, D = 2, 2048, 1024
H, HS = 16, 64
THETA = 10000.0
NCORES = 8
HG = 4            # heads per core
HD = HG * HS      # head dims per core = 256
WS = 32.0          # host-side q/k weight scale (fp8 subnormal avoidance)
SCALE = 0.125 / (WS * WS)  # exp scale: 1/sqrt(HS) / (q,k weight scales)
NEG = -1.0e5       # additive mask (exp underflows to 0)

_NC = None


_SELF_SEM = {
    "EngineType.Activation": "Activation_",
    "EngineType.DVE": "DVE_",
    "EngineType.PE": "PE_",
    "EngineType.Pool": "Pool_",
}


def _split_multi_waits(nc):
    """walrus codegen accepts at most ONE semaphore wait per engine
    instruction (the 64B ISA structs have a single EVENTS slot); Tile's
    scheduler freely emits several.  Hoist all but the last wait of each
    instruction onto inserted same-engine EventSemaphore (poll_sem) ops,
    which preserves semantics exactly (engines execute sequentially).

    Additionally drop ge-waits on the instruction's OWN engine semaphore
    for compute engines: those guard WAW/WAR against earlier instructions
    of the same in-order engine, which program order already guarantees."""
    def _names(args):
        out = set()
        for a in args:
            for attr in ("memref", "name"):
                v = getattr(a, attr, None)
                if isinstance(v, str):
                    out.add(v.removesuffix("_set"))
            t = getattr(a, "tensor", None)
            if t is not None and isinstance(getattr(t, "name", None), str):
                out.add(t.name)
        return out

    eng_written = {}
    eng_read = {}
    _COMPUTE = {"InstActivation", "InstTensorTensor", "InstTensorCopy",
                "InstMatmult", "InstLdweights", "InstMemset",
                "InstTensorScalarPtr", "InstTensorReduce"}
    for f in nc.m.functions:
        for blk in f.blocks:
            for inst in blk.instructions:
                if type(inst).__name__ in _COMPUTE:
                    e = str(inst.engine)
                    eng_written.setdefault(e, set()).update(_names(inst.outs))
                    eng_read.setdefault(e, set()).update(_names(inst.ins))

    n = 0
    for f in nc.m.functions:
        for blk in f.blocks:
            il = blk.instructions
            i = 0
            while i < len(il):
                inst = il[i]
                si = inst.sync_info
                if si is None or not si.on_wait:
                    i += 1
                    continue
                waits = list(si.on_wait)
                eng = str(inst.engine)
                selfpfx = _SELF_SEM.get(eng)
                if (selfpfx is not None
                        and type(inst).__name__ in (
                            "InstActivation", "InstMatmult", "InstLdweights",
                            "InstTensorTensor", "InstTensorCopy", "InstMemset")
                        and not (_names(inst.ins) & eng_written.get(eng, set()))
                        and not (_names(inst.outs) & eng_read.get(eng, set()))):
                    kept = [w for w in waits
                            if not (w.wait_mode == "sem-ge-imm"
                                    and w.ant_name.startswith(selfpfx))]
                    if len(kept) != len(waits):
                        waits = kept
                        inst.sync_info = mybir.SyncInfo(
                            on_wait=waits, on_update=list(si.on_update))
                if len(waits) > 1:
                    for w in waits[:-1]:
                        es = mybir.InstEventSemaphore(name=f"I-wsplit-{n}")
                        n += 1
                        es.engine = inst.engine
                        es.sync_info = mybir.SyncInfo(on_wait=[w], on_update=[])
                        nc.register_instruction(es)
                        il.insert(i, es)
                        i += 1
                    inst.sync_info = mybir.SyncInfo(
                        on_wait=[waits[-1]], on_update=list(si.on_update))
                i += 1
    return n


def _dedup_ldweights(nc):
    """bass emits one InstLdweights per InstMatmult.  When a later
    InstLdweights loads the IDENTICAL weights AP that is already resident
    in the PE array (no other InstLdweights in between), the reload is
    redundant: MATMUL does not self-load for 16-bit dtypes.  Delete it,
    folding its waits into the following matmult."""
    def fp(inst):
        a = inst.ins[0]
        return (a.memref, a.offset, str(a.ap), str(a.dtype),
                str(getattr(inst, "perf_mode", None)))

    n = 0
    for f in nc.m.functions:
        for blk in f.blocks:
            il = blk.instructions
            last = None
            i = 0
            while i < len(il):
                inst = il[i]
                tn = type(inst).__name__
                if tn == "InstLdweights":
                    cur = fp(inst)
                    si = inst.sync_info
                    if cur == last and (si is None or not si.on_update):
                        waits = list(si.on_wait) if si is not None else []
                        if waits:
                            j = i + 1
                            while (j < len(il)
                                   and type(il[j]).__name__ != "InstMatmult"):
                                j += 1
                            if j == len(il):
                                i += 1
                                continue
                            mm = il[j]
                            msi = mm.sync_info
                            mw = list(msi.on_wait) if msi is not None else []
                            mu = list(msi.on_update) if msi is not None else []
                            mm.sync_info = mybir.SyncInfo(
                                on_wait=waits + mw, on_update=mu)
                        del il[i]
                        n += 1
                        continue
                    last = cur
                i += 1
    return n


def build_nc():
    nc = bass.Bass()

    xT8 = nc.declare_dram_parameter("xT8", [D, T], FP8, isOutput=False)
    xT = nc.declare_dram_parameter("xT", [D, T], BF16, isOutput=False)
    wq = nc.declare_dram_parameter("wq", [D, HD], FP8, isOutput=False)
    wk = nc.declare_dram_parameter("wk", [D, HD], FP8, isOutput=False)
    wv = nc.declare_dram_parameter("wv", [D, HD], BF16, isOutput=False)
    wo = nc.declare_dram_parameter("wo", [HD, D], BF16, isOutput=False)
    # [bq(256) | bk(256)] as a single stationary bias row (scaled by WS)
    brow = nc.declare_dram_parameter("brow", [1, 2 * HD], BF16, isOutput=False)
    bvrow = nc.declare_dram_parameter("bvrow", [1, HD], BF16, isOutput=False)
    cos2 = nc.declare_dram_parameter("cos2", [128, T], BF16, isOutput=False)
    sin2 = nc.declare_dram_parameter("sin2", [128, T], BF16, isOutput=False)
    # -1e5 * strict upper triangle (stationary for the diag mask matmul)
    maskt = nc.declare_dram_parameter("maskt", [128, 128], BF16, isOutput=False)
    # identity repeated twice (moving for the diag mask matmul)
    id2 = nc.declare_dram_parameter("id2", [128, 256], BF16, isOutput=False)
    kb = nc.declare_dram_parameter("kb", [T], F32, isOutput=False)
    out = nc.declare_dram_parameter("out", [T, D], BF16, isOutput=True)

    NK = T // 128   # 16 key tiles

    with tile.TileContext(nc) as tc:
        with (
            tc.tile_pool(name="const", bufs=1) as cpool,
            tc.tile_pool(name="xw", bufs=1) as xwpool,
            tc.tile_pool(name="qk", bufs=1) as qkpool,
            tc.tile_pool(name="raw", bufs=3) as rawpool,
            tc.tile_pool(name="p", bufs=6) as ppool,
            tc.tile_pool(name="rec", bufs=2) as rpool,
            tc.tile_pool(name="ev", bufs=3) as evpool,
            tc.tile_pool(name="psP", bufs=2, space="PSUM") as psP,
            tc.tile_pool(name="psS", bufs=2, space="PSUM") as psS,
            tc.tile_pool(name="psY", bufs=1, space="PSUM") as psY,
        ):
            # ---- constant / weight loads ----
            # wq + the first xT8 chunks gate the first matmul groups; x is
            # loaded in 512-col chunks so projections start after ~0.5MB.
            wq_sb = xwpool.tile([128, 4, 2, HD], FP8, tag="wq")
            wk_sb = xwpool.tile([128, 4, 2, HD], FP8, tag="wk")
            wv_sb = xwpool.tile([128, 8, HD], BF16, tag="wv")
            wo_sb = xwpool.tile([128, 2, D], BF16, tag="wo")
            brow_sb = cpool.tile([1, 2 * HD], BF16, tag="brow")
            bvrow_sb = cpool.tile([1, HD], BF16, tag="bvrow")
            ones_sb = cpool.tile([1, 512], BF16, tag="ones")
            wq_r = wq.ap().rearrange("(i j p) n -> p i j n", p=128, j=2)
            wk_r = wk.ap().rearrange("(i j p) n -> p i j n", p=128, j=2)
            # pair-0 (c2=0) weight halves first: the first attention quarter
            # needs only these
            nc.sync.dma_start(wq_sb[:, :, :, 0:128], wq_r[:, :, :, 0:128])
            nc.sync.dma_start(brow_sb[:], brow.ap())
            nc.sync.dma_start(bvrow_sb[:], bvrow.ap())
            nc.vector.memset(ones_sb[:], 1.0)

            # fp8 x (q/k projections, DoubleRow pairs) + bf16 x (V projection)
            xt8s = [xwpool.tile([128, 2, T], FP8, tag=f"x8{i}", name=f"x8{i}")
                    for i in range(4)]
            xts = [xwpool.tile([128, T], BF16, tag=f"xt{dc}", name=f"xt{dc}")
                   for dc in range(8)]

            def load_xt_tr(tr):
                lo, hi = tr * 512, (tr + 1) * 512
                for i in range(4):
                    for j in range(2):
                        nc.sync.dma_start(
                            xt8s[i][:, j, lo:hi],
                            xT8[(2 * i + j) * 128:(2 * i + j + 1) * 128, lo:hi],
                        )
                for dc in range(8):
                    nc.sync.dma_start(
                        xts[dc][:, lo:hi],
                        xT[dc * 128:(dc + 1) * 128, lo:hi],
                    )

            cos_sb = cpool.tile([128, T], BF16, tag="cos")
            sin_sb = cpool.tile([128, T], BF16, tag="sin")
            maskt_sb = cpool.tile([128, 128], BF16, tag="maskt")
            id2_sb = cpool.tile([128, 2, 128], BF16, tag="id2")
            kb_sb = cpool.tile([128, NK], F32, tag="kb")
            load_xt_tr(0)
            nc.sync.dma_start(wk_sb[:, :, :, 0:128], wk_r[:, :, :, 0:128])
            nc.sync.dma_start(cos_sb[:, 0:512], cos2[:, 0:512])
            nc.sync.dma_start(sin_sb[:, 0:512], sin2[:, 0:512])
            nc.sync.dma_start(wv_sb[:], wv.ap().rearrange("(c p) n -> p c n", p=128))
            nc.sync.dma_start(maskt_sb[:], maskt.ap())
            nc.sync.dma_start(
                id2_sb[:], id2.ap().rearrange("p (two q) -> p two q", two=2))
            nc.sync.dma_start(kb_sb[:], kb.ap().rearrange("(t p) -> p t", p=128))
            load_xt_tr(1)
            nc.sync.dma_start(cos_sb[:, 512:T], cos2[:, 512:T])
            nc.sync.dma_start(sin_sb[:, 512:T], sin2[:, 512:T])
            load_xt_tr(2)
            load_xt_tr(3)
            nc.sync.dma_start(wq_sb[:, :, :, 128:256], wq_r[:, :, :, 128:256])
            nc.sync.dma_start(wk_sb[:, :, :, 128:256], wk_r[:, :, :, 128:256])
            nc.sync.dma_start(wo_sb[:], wo.ap().rearrange("(c p) n -> p c n", p=128))

            # persistent [128, T] tiles: 2 heads each (rows 0:64 / 64:128)
            qT = [qkpool.tile([128, T], BF16, tag=f"qT{c}", name=f"qT{c}") for c in range(2)]
            kT = [qkpool.tile([128, T], BF16, tag=f"kT{c}", name=f"kT{c}") for c in range(2)]
            yT = [qkpool.tile([128, T], BF16, tag=f"yT{c}", name=f"yT{c}") for c in range(2)]

            # ---- Q^T / K^T projection (fp8 DoubleRow) + RoPE, 512-col chunk
            # ti: 0 = q, 1 = k (selects bias row slice)
            def proj_qk_chunk(ti, wsb, c2, fin, tr):
                lo, hi = tr * 512, (tr + 1) * 512
                ps = psP.tile([128, 512], F32, tag="pp")
                for i in range(4):
                    nc.tensor.matmul(
                        ps[:],
                        wsb[:, i, :, c2 * 128:(c2 + 1) * 128],
                        xt8s[i][:, :, lo:hi],
                        start=(i == 0),
                        stop=False,
                        perf_mode=DR,
                    )
                # bias via rank-1 update: bias row x ones row
                nc.tensor.matmul(
                    ps[:],
                    brow_sb[0:1, ti * HD + c2 * 128: ti * HD + (c2 + 1) * 128],
                    ones_sb[0:1, :],
                    start=False, stop=True,
                )
                raw = rawpool.tile([128, 512], BF16, tag="raw")
                nc.vector.tensor_copy(raw[:], ps[:])
                # RoPE: fin = raw*cos + rot(raw)*sin_signed
                f = fin
                for (do, di) in ((0, 32), (32, 0), (64, 96), (96, 64)):
                    nc.vector.tensor_copy(f[do:do + 32, lo:hi], raw[di:di + 32, :])
                nc.vector.tensor_mul(f[:, lo:hi], f[:, lo:hi], sin_sb[:, lo:hi])
                nc.vector.tensor_mul(raw[:], raw[:], cos_sb[:, lo:hi])
                # final add on Pool (idle) to unload DVE
                nc.gpsimd.tensor_add(f[:, lo:hi], f[:, lo:hi], raw[:])

            # same projection for TWO 512-col chunks, subtile-major over two
            # psum tiles: each stationary slice serves both chunks so
            # _dedup_ldweights deletes every second (identical) weight load.
            def proj_qk_pair(ti, wsb, c2, fin, trp):
                trs = (2 * trp, 2 * trp + 1)
                pss = [psP.tile([128, 512], F32, tag="pp", name=f"pp{t}")
                       for t in range(2)]
                for i in range(4):
                    for t in range(2):
                        lo = trs[t] * 512
                        nc.tensor.matmul(
                            pss[t][:],
                            wsb[:, i, :, c2 * 128:(c2 + 1) * 128],
                            xt8s[i][:, :, lo:lo + 512],
                            start=(i == 0),
                            stop=False,
                            perf_mode=DR,
                            skip_group_check=True,
                        )
                for t in range(2):
                    nc.tensor.matmul(
                        pss[t][:],
                        brow_sb[0:1, ti * HD + c2 * 128: ti * HD + (c2 + 1) * 128],
                        ones_sb[0:1, :],
                        start=False, stop=True,
                        skip_group_check=True,
                    )
                for t in range(2):
                    lo = trs[t] * 512
                    hi = lo + 512
                    raw = rawpool.tile([128, 512], BF16, tag="raw")
                    nc.vector.tensor_copy(raw[:], pss[t][:])
                    f = fin
                    for (do, di) in ((0, 32), (32, 0), (64, 96), (96, 64)):
                        nc.vector.tensor_copy(f[do:do + 32, lo:hi], raw[di:di + 32, :])
                    nc.vector.tensor_mul(f[:, lo:hi], f[:, lo:hi], sin_sb[:, lo:hi])
                    nc.vector.tensor_mul(raw[:], raw[:], cos_sb[:, lo:hi])
                    nc.gpsimd.tensor_add(f[:, lo:hi], f[:, lo:hi], raw[:])

            # ---- V projection (bf16, bias as rank-1, ones denominator cols)
            vts = [None] * NK

            def proj_v(kt):
                ps = psP.tile([128, HD], F32, tag="pp")
                for dc in range(8):
                    nc.tensor.matmul(
                        ps[:],
                        xts[dc][:, kt * 128:(kt + 1) * 128],
                        wv_sb[:, dc, :],
                        start=(dc == 0),
                        stop=False,
                    )
                nc.tensor.matmul(
                    ps[:], ones_sb[0:1, 0:128], bvrow_sb[0:1, :],
                    start=False, stop=True,
                )
                vt = xwpool.tile([128, HG, 128], BF16, tag=f"v{kt}", name=f"v{kt}")
                nc.vector.tensor_copy(
                    vt[:, :, 0:64],
                    ps[:].rearrange("p (h d) -> p h d", h=HG),
                )
                nc.vector.memset(vt[:, :, 64:128], 1.0)
                vts[kt] = vt

            # ---- attention for head pair c2, one q-quarter (512 cols) ----
            # scores/probs/Y for both heads side by side in one [128, 2, 512]
            # tile: [:, 0, :] = head 2*c2, [:, 1, :] = 2*c2+1.
            def attn_quarter(c2, qq):
                qlo = qq * 512
                last = 4 * qq + 3
                y = psY.tile([128, 2, 512], F32, tag="y", name=f"y{c2}_{qq}")
                for kt in range(last + 1):
                    j = kt - 4 * qq
                    c = j * 128 if j >= 0 else 0   # first valid col (diag trim)
                    ksl = slice(kt * 128, (kt + 1) * 128)
                    qsl = slice(qlo + c, qlo + 512)
                    s = psS.tile([128, 2, 512], F32, tag="s")
                    # two row-group-concurrent 64-row score matmuls
                    nc.tensor.matmul(
                        s[:, 0, c:], kT[c2][0:64, ksl], qT[c2][0:64, qsl],
                        start=True, stop=True,
                    )
                    nc.tensor.matmul(
                        s[:, 1, c:], kT[c2][64:128, ksl], qT[c2][64:128, qsl],
                        start=True, stop=True,
                    )
                    if j >= 0:
                        # diagonal block: add -1e5 strict upper triangle for
                        # both heads (exp then underflows to 0)
                        nc.tensor.matmul(
                            s[:, :, c:c + 128], maskt_sb[:], id2_sb[:],
                            start=False, stop=True, skip_group_check=True,
                        )
                    p = ppool.tile([128, 2, 512], BF16, tag="p")
                    nc.scalar.activation(
                        p[:, :, c:], s[:, :, c:],
                        mybir.ActivationFunctionType.Exp,
                        bias=kb_sb[:, kt:kt + 1], scale=SCALE,
                    )
                    for h in (0, 1):
                        nc.tensor.matmul(
                            y[:, h, c:],
                            vts[kt][:, 2 * c2 + h, :],
                            p[:, h, c:],
                            start=(kt == 0),
                            stop=(kt == last),
                            skip_group_check=True,
                        )
                # normalize both heads at once: 1/r = exp(-ln r) (ln and exp
                # share an ACT table so no table reload)
                lnr = rpool.tile([64, 2, 512], F32, tag="lnr")
                rec = rpool.tile([64, 2, 512], F32, tag="rec")
                nc.scalar.activation(
                    lnr[:], y[64:128, :, :], mybir.ActivationFunctionType.Ln)
                nc.scalar.activation(
                    rec[:], lnr[:], mybir.ActivationFunctionType.Exp,
                    scale=-1.0)
                nc.vector.tensor_mul(
                    yT[c2][0:64, qlo:qlo + 512], y[0:64, 0, :], rec[:, 0, :])
                nc.vector.tensor_mul(
                    yT[c2][64:128, qlo:qlo + 512], y[0:64, 1, :], rec[:, 1, :])

            def outproj_quarter(qq):
                # partial out for T-tiles of this quarter; host adds bo+reduces
                # c2-outer / dr-inner: each yT stationary slice serves both wo
                # column halves (second load deleted by _dedup_ldweights).
                # Last two quarters evict on ScalarE: the exp stream is done
                # by then, and it shortens the DVE-bound tail.
                for tt in range(4 * qq, 4 * qq + 4):
                    outproj_tt(tt, qq >= 2)

            def outproj_tt(tt, on_scalar):
                pss = [psP.tile([128, 512], F32, tag="pp", name=f"po{t}")
                       for t in range(2)]
                for c2 in range(2):
                    for dr in range(2):
                        nc.tensor.matmul(
                            pss[dr][:],
                            yT[c2][:, tt * 128:(tt + 1) * 128],
                            wo_sb[:, c2, dr * 512:(dr + 1) * 512],
                            start=(c2 == 0),
                            stop=(c2 == 1),
                            skip_group_check=True,
                        )
                for dr in range(2):
                    ev = evpool.tile([128, 512], BF16, tag="ev")
                    if on_scalar:
                        nc.scalar.activation(
                            ev[:], pss[dr][:],
                            mybir.ActivationFunctionType.Identity)
                    else:
                        nc.vector.tensor_copy(ev[:], pss[dr][:])
                    nc.sync.dma_start(
                        out[tt * 128:(tt + 1) * 128, dr * 512:(dr + 1) * 512],
                        ev[:],
                    )

            # ---- emission order == scheduler priority ----
            # quarter qq of pair 0 needs exactly q/k chunks tr<=qq and
            # V tiles kt<=4qq+3, so interleave per quarter.
            for qq in range(2):
                proj_qk_chunk(0, wq_sb, 0, qT[0], qq)
                proj_qk_chunk(1, wk_sb, 0, kT[0], qq)
                for kt in range(4 * qq, 4 * qq + 4):
                    proj_v(kt)
                attn_quarter(0, qq)
            proj_qk_pair(0, wq_sb, 0, qT[0], 1)
            proj_qk_pair(1, wk_sb, 0, kT[0], 1)
            for kt in range(8, 12):
                proj_v(kt)
            attn_quarter(0, 2)
            for kt in range(12, 16):
                proj_v(kt)
            attn_quarter(0, 3)
            proj_qk_pair(0, wq_sb, 1, qT[1], 0)
            proj_qk_pair(1, wk_sb, 1, kT[1], 0)
            attn_quarter(1, 0)
            proj_qk_pair(0, wq_sb, 1, qT[1], 1)
            attn_quarter(1, 1)
            proj_qk_pair(1, wk_sb, 1, kT[1], 1)
            outproj_quarter(0)
            attn_quarter(1, 2)
            outproj_quarter(1)
            attn_quarter(1, 3)
            outproj_quarter(2)
            outproj_quarter(3)
    nd = _dedup_ldweights(nc)
    _split_multi_waits(nc)
    assert nd > 0, f"expected ldweights dedup to fire, got {nd}"
    return nc


def _rope_tables():
    inv_freq = 1.0 / (THETA ** (np.arange(0, HS, 2, dtype=np.float64) / HS))  # [32]
    t = np.arange(T, dtype=np.float64)
    fr = t[:, None] * inv_freq[None, :]          # [T, 32]
    emb = np.concatenate([fr, fr], axis=1)       # [T, 64]
    cos = np.cos(emb).T.astype(np.float32)       # [64, T]
    sin = np.sin(emb).T.astype(np.float32)       # [64, T]
    sin_signed = sin.copy()
    sin_signed[0:32] = -sin_signed[0:32]
    cos2 = np.concatenate([cos, cos], axis=0)            # [128, T]
    sin2 = np.concatenate([sin_signed, sin_signed], 0)   # [128, T]
    return cos2.astype(ml_dtypes.bfloat16), sin2.astype(ml_dtypes.bfloat16)


def _in_maps(x, attention_mask, Wq, bqv, Wk, bkv, Wv, bvv, Wo):
    cos2, sin2 = _rope_tables()
    maskt = (NEG * np.triu(np.ones((128, 128), np.float32), k=1)
             ).astype(ml_dtypes.bfloat16)
    id2 = np.concatenate([np.eye(128, dtype=np.float32)] * 2, axis=1
                         ).astype(ml_dtypes.bfloat16)
    f8 = ml_dtypes.float8_e4m3
    bf = ml_dtypes.bfloat16
    xTs = [np.ascontiguousarray(x[b].T) for b in range(B)]
    xT8s = [xt.astype(f8) for xt in xTs]
    xT16s = [xt.astype(bf) for xt in xTs]
    kbs = [
        np.where(attention_mask[b] != 0, 0.0, NEG).astype(np.float32)
        for b in range(B)
    ]
    maps = []
    for core in range(NCORES):
        b, g = core // 4, core % 4
        sl = slice(g * HD, (g + 1) * HD)
        brow = (np.concatenate([bqv[sl], bkv[sl]]) * WS).reshape(1, 2 * HD)
        maps.append({
            "xT8": xT8s[b],
            "xT": xT16s[b],
            "wq": np.ascontiguousarray(Wq[:, sl] * WS).astype(f8),
            "wk": np.ascontiguousarray(Wk[:, sl] * WS).astype(f8),
            "wv": np.ascontiguousarray(Wv[:, sl]).astype(bf),
            "wo": np.ascontiguousarray(Wo[sl, :]).astype(bf),
            "brow": brow.astype(bf),
            "bvrow": bvv[sl].astype(bf).reshape(1, HD),
            "cos2": cos2,
            "sin2": sin2,
            "maskt": maskt,
            "id2": id2,
            "kb": kbs[b],
        })
    return maps


def _run(inputs, trace=False):
    global _NC
    if _NC is None:
        _NC = build_nc()
    maps = _in_maps(
        np.asarray(inputs["x"]), np.asarray(inputs["attention_mask"]),
        np.asarray(inputs["Wq"]), np.asarray(inputs["bq"]),
        np.asarray(inputs["Wk"]), np.asarray(inputs["bk"]),
        np.asarray(inputs["Wv"]), np.asarray(inputs["bv"]),
        np.asarray(inputs["Wo"]),
    )
    res = run_bass_kernel_spmd(_NC, maps, core_ids=list(range(NCORES)), trace=trace)
    bo = np.asarray(inputs["bo"], np.float32)
    outs = []
    for b in range(B):
        acc = np.zeros((T, D), np.float32)
        for g in range(4):
            acc += np.asarray(res.results[b * 4 + g]["out"], np.float32)
        outs.append(acc + bo[None, :])
    return np.stack(outs, axis=0), res


def kernel(**inputs):
    out, _ = _run(inputs, trace=False)
    return out


# revision 5
# speedup vs baseline: 1.2171x; 1.2107x over previous
"""Distributed Trainium2 Bass kernel for a 16-head causal RoPE attention layer.

Problem: B=2, T=2048, D=1024, H=16, HS=64 (fp32 reference).

Sharding (8 cores): core = b*4 + g, b in {0,1} (batch), g in {0..3} (group of
4 heads).  Each core computes Q/K/V projections for its 256 head-dims, runs
causal attention for its 4 heads, and applies its 256-row slice of Wo,
producing a partial [T, D] output.  The host sums the 4 partials per batch
and adds bo.  No on-device collectives.

v5: all-bf16 compute (fp8 q/k measured 1.7e-2 rel err from softmax weight
noise -- too close to the 2e-2 gate), with the time recovered by keeping
the PE dense (Trainium2's PE downclocks 2.4->1.2GHz after any pipeline
gap and needs ~3us of continuous work to ramp back):
  - attention Y matmuls are software-pipelined TWO key-tiles behind their
    scores, so Y(kt-2) never waits on exp(kt-2) (it finished during the
    scores of kt-1/kt) and the PE stream has no per-kt dependency stall.
  - small independent PE units (projection chunks, V tiles, outproj
    T-tiles) are scattered one-per-two-kt-steps inside the attention
    quarters as fillers, sized so the per-kt PE time tracks the ~1.2us
    exp op and the ACT stream is never starved of scores.
  - q/k biases ride the PSUM eviction as a DVE tensor_scalar add (bias is
    per-partition in the q^T layout) -- no rank-1 bias matmuls.
  - causal masking: the diagonal 128x128 strict upper triangle is added
    as a -4e5 stationary matmul into the scores psum (PE), so exp
    underflows to 0 and DVE never touches the exp->Y path.
  - V bias via rank-1 matmul; normalize via ln/exp on ACT (shared table).
ACT is the bottleneck (~90us exp + ~21us ln/exp normalize); PE (~105us)
runs just under it when dense, DVE ~85us, Pool ~23us (RoPE adds).
PSUM: scores 2x2 banks + Y 2 banks + proj/outproj ring 2x1 = 8 exactly.
"""

import numpy as np
import ml_dtypes

import concourse.bass as bass
import concourse.mybir as mybir
import concourse.tile as tile
from concourse.bass_utils import run_bass_kernel_spmd

BF16 = mybir.dt.bfloat16
F32 = mybir.dt.float32

B, T, D = 2, 2048, 1024
H, HS = 16, 64
THETA = 10000.0
NCORES = 8
HG = 4            # heads per core
HD = HG * HS      # head dims per core = 256
SCALE = 1.0 / 8.0  # 1/sqrt(HS)
NEG = -1.0e5       # additive mask for padded keys (exp underflows to 0)
NEGM = -4.0e5      # diagonal-mask matmul constant (survives bf16 p exactly as 0)

_NC = None


_SELF_SEM = {
    "EngineType.Activation": "Activation_",
    "EngineType.DVE": "DVE_",
    "EngineType.PE": "PE_",
    "EngineType.Pool": "Pool_",
}


def _split_multi_waits(nc):
    """walrus codegen accepts at most ONE semaphore wait per engine
    instruction (the 64B ISA structs have a single EVENTS slot); Tile's
    scheduler freely emits several.  Hoist all but the last wait of each
    instruction onto inserted same-engine EventSemaphore (poll_sem) ops,
    which preserves semantics exactly (engines execute sequentially).

    Additionally drop ge-waits on the instruction's OWN engine semaphore
    for compute engines: those guard WAW/WAR against earlier instructions
    of the same in-order engine, which program order already guarantees."""
    def _names(args):
        out = set()
        for a in args:
            for attr in ("memref", "name"):
                v = getattr(a, attr, None)
                if isinstance(v, str):
                    out.add(v.removesuffix("_set"))
            t = getattr(a, "tensor", None)
            if t is not None and isinstance(getattr(t, "name", None), str):
                out.add(t.name)
        return out

    eng_written = {}
    eng_read = {}
    _COMPUTE = {"InstActivation", "InstTensorTensor", "InstTensorCopy",
                "InstMatmult", "InstLdweights", "InstMemset",
                "InstTensorScalarPtr", "InstTensorReduce"}
    for f in nc.m.functions:
        for blk in f.blocks:
            for inst in blk.instructions:
                if type(inst).__name__ in _COMPUTE:
                    e = str(inst.engine)
                    eng_written.setdefault(e, set()).update(_names(inst.outs))
                    eng_read.setdefault(e, set()).update(_names(inst.ins))

    n = 0
    for f in nc.m.functions:
        for blk in f.blocks:
            il = blk.instructions
            i = 0
            while i < len(il):
                inst = il[i]
                si = inst.sync_info
                if si is None or not si.on_wait:
                    i += 1
                    continue
                waits = list(si.on_wait)
                eng = str(inst.engine)
                selfpfx = _SELF_SEM.get(eng)
                if (selfpfx is not None
                        and type(inst).__name__ in (
                            "InstActivation", "InstMatmult", "InstLdweights",
                            "InstTensorTensor", "InstTensorCopy", "InstMemset")
                        and not (_names(inst.ins) & eng_written.get(eng, set()))
                        and not (_names(inst.outs) & eng_read.get(eng, set()))):
                    kept = [w for w in waits
                            if not (w.wait_mode == "sem-ge-imm"
                                    and w.ant_name.startswith(selfpfx))]
                    if len(kept) != len(waits):
                        waits = kept
                        inst.sync_info = mybir.SyncInfo(
                            on_wait=waits, on_update=list(si.on_update))
                if len(waits) > 1:
                    for w in waits[:-1]:
                        es = mybir.InstEventSemaphore(name=f"I-wsplit-{n}")
                        n += 1
                        es.engine = inst.engine
                        es.sync_info = mybir.SyncInfo(on_wait=[w], on_update=[])
                        nc.register_instruction(es)
                        il.insert(i, es)
                        i += 1
                    inst.sync_info = mybir.SyncInfo(
                        on_wait=[waits[-1]], on_update=list(si.on_update))
                i += 1
    return n


def _dedup_ldweights(nc):
    """bass emits one InstLdweights per InstMatmult.  When a later
    InstLdweights loads the IDENTICAL weights AP that is already resident
    in the PE array (no other InstLdweights in between), the reload is
    redundant: MATMUL does not self-load for 16-bit dtypes.  Delete it,
    folding its waits into the following matmult."""
    def fp(inst):
        a = inst.ins[0]
        return (a.memref, a.offset, str(a.ap), str(a.dtype),
                str(getattr(inst, "perf_mode", None)))

    n = 0
    for f in nc.m.functions:
        for blk in f.blocks:
            il = blk.instructions
            last = None
            i = 0
            while i < len(il):
                inst = il[i]
                tn = type(inst).__name__
                if tn == "InstLdweights":
                    cur = fp(inst)
                    si = inst.sync_info
                    if cur == last and (si is None or not si.on_update):
                        waits = list(si.on_wait) if si is not None else []
                        if waits:
                            j = i + 1
                            while (j < len(il)
                                   and type(il[j]).__name__ != "InstMatmult"):
                                j += 1
                            if j == len(il):
                                i += 1
                                continue
                            mm = il[j]
                            msi = mm.sync_info
                            mw = list(msi.on_wait) if msi is not None else []
                            mu = list(msi.on_update) if msi is not None else []
                            mm.sync_info = mybir.SyncInfo(
                                on_wait=waits + mw, on_update=mu)
                        del il[i]
                        n += 1
                        continue
                    last = cur
                i += 1
    return n


def build_nc():
    nc = bass.Bass()

    xT = nc.declare_dram_parameter("xT", [D, T], BF16, isOutput=False)
    wq = nc.declare_dram_parameter("wq", [D, HD], BF16, isOutput=False)
    wk = nc.declare_dram_parameter("wk", [D, HD], BF16, isOutput=False)
    wv = nc.declare_dram_parameter("wv", [D, HD], BF16, isOutput=False)
    wo = nc.declare_dram_parameter("wo", [HD, D], BF16, isOutput=False)
    # per-partition bias columns: [:, 2*ti+c2] = bias for q/k (ti) pair c2
    bcol = nc.declare_dram_parameter("bcol", [128, 4], F32, isOutput=False)
    bvrow = nc.declare_dram_parameter("bvrow", [1, HD], BF16, isOutput=False)
    cos2 = nc.declare_dram_parameter("cos2", [128, T], BF16, isOutput=False)
    sin2 = nc.declare_dram_parameter("sin2", [128, T], BF16, isOutput=False)
    # NEGM * strict upper triangle (stationary for the diag mask matmul)
    maskt = nc.declare_dram_parameter("maskt", [128, 128], BF16, isOutput=False)
    # identity repeated twice (moving for the diag mask matmul)
    id2 = nc.declare_dram_parameter("id2", [128, 256], BF16, isOutput=False)
    kb = nc.declare_dram_parameter("kb", [T], F32, isOutput=False)
    out = nc.declare_dram_parameter("out", [T, D], BF16, isOutput=True)

    NK = T // 128   # 16 key tiles

    with tile.TileContext(nc) as tc:
        with (
            tc.tile_pool(name="const", bufs=1) as cpool,
            tc.tile_pool(name="xw", bufs=1) as xwpool,
            tc.tile_pool(name="qk", bufs=1) as qkpool,
            tc.tile_pool(name="raw", bufs=3) as rawpool,
            tc.tile_pool(name="p", bufs=6) as ppool,
            tc.tile_pool(name="rec", bufs=2) as rpool,
            tc.tile_pool(name="ev", bufs=3) as evpool,
            tc.tile_pool(name="psP", bufs=2, space="PSUM") as psP,
            tc.tile_pool(name="psS", bufs=2, space="PSUM") as psS,
            tc.tile_pool(name="psY", bufs=1, space="PSUM") as psY,
        ):
            # ---- constant / weight loads ----
            # wq + the first xT column-chunks gate the first matmul groups;
            # xT is loaded in 512-col chunks so projections start early.
            wq_sb = xwpool.tile([128, 8, HD], BF16, tag="wq")
            wk_sb = xwpool.tile([128, 8, HD], BF16, tag="wk")
            wv_sb = xwpool.tile([128, 8, HD], BF16, tag="wv")
            wo_sb = xwpool.tile([128, 2, D], BF16, tag="wo")
            bcol_sb = cpool.tile([128, 4], F32, tag="bcol")
            bvrow_sb = cpool.tile([1, HD], BF16, tag="bvrow")
            ones_sb = cpool.tile([1, 128], BF16, tag="ones")
            wq_r = wq.ap().rearrange("(c p) n -> p c n", p=128)
            wk_r = wk.ap().rearrange("(c p) n -> p c n", p=128)
            # pair-0 (c2=0) weight halves first: the first attention quarter
            # needs only these
            nc.sync.dma_start(wq_sb[:, :, 0:128], wq_r[:, :, 0:128])
            nc.sync.dma_start(bcol_sb[:], bcol.ap())
            nc.sync.dma_start(bvrow_sb[:], bvrow.ap())
            nc.vector.memset(ones_sb[:], 1.0)

            xts = []
            for dc in range(8):
                xt = xwpool.tile([128, T], BF16, tag=f"xt{dc}", name=f"xt{dc}")
                xts.append(xt)

            def load_xt_tr(tr):
                for dc in range(8):
                    nc.sync.dma_start(
                        xts[dc][:, tr * 512:(tr + 1) * 512],
                        xT[dc * 128:(dc + 1) * 128, tr * 512:(tr + 1) * 512],
                    )

            cos_sb = cpool.tile([128, T], BF16, tag="cos")
            sin_sb = cpool.tile([128, T], BF16, tag="sin")
            maskt_sb = cpool.tile([128, 128], BF16, tag="maskt")
            id2_sb = cpool.tile([128, 2, 128], BF16, tag="id2")
            kb_sb = cpool.tile([128, NK], F32, tag="kb")
            load_xt_tr(0)
            nc.sync.dma_start(wk_sb[:, :, 0:128], wk_r[:, :, 0:128])
            nc.sync.dma_start(cos_sb[:, 0:512], cos2[:, 0:512])
            nc.sync.dma_start(sin_sb[:, 0:512], sin2[:, 0:512])
            nc.sync.dma_start(wv_sb[:], wv.ap().rearrange("(c p) n -> p c n", p=128))
            nc.sync.dma_start(maskt_sb[:], maskt.ap())
            nc.sync.dma_start(
                id2_sb[:], id2.ap().rearrange("p (two q) -> p two q", two=2))
            nc.sync.dma_start(kb_sb[:], kb.ap().rearrange("(t p) -> p t", p=128))
            load_xt_tr(1)
            nc.sync.dma_start(cos_sb[:, 512:T], cos2[:, 512:T])
            nc.sync.dma_start(sin_sb[:, 512:T], sin2[:, 512:T])
            load_xt_tr(2)
            load_xt_tr(3)
            nc.sync.dma_start(wq_sb[:, :, 128:256], wq_r[:, :, 128:256])
            nc.sync.dma_start(wk_sb[:, :, 128:256], wk_r[:, :, 128:256])
            nc.sync.dma_start(wo_sb[:], wo.ap().rearrange("(c p) n -> p c n", p=128))

            # persistent [128, T] tiles: 2 heads each (rows 0:64 / 64:128)
            qT = [qkpool.tile([128, T], BF16, tag=f"qT{c}", name=f"qT{c}") for c in range(2)]
            kT = [qkpool.tile([128, T], BF16, tag=f"kT{c}", name=f"kT{c}") for c in range(2)]
            yT = [qkpool.tile([128, T], BF16, tag=f"yT{c}", name=f"yT{c}") for c in range(2)]

            # ---- Q^T / K^T projection + RoPE, one 512-col chunk ----
            # ti: 0 = q, 1 = k (selects bias column)
            def proj_qk_chunk(ti, wsb, c2, fin, tr):
                lo, hi = tr * 512, (tr + 1) * 512
                ps = psP.tile([128, 512], F32, tag="pp")
                for dc in range(8):
                    nc.tensor.matmul(
                        ps[:],
                        wsb[:, dc, c2 * 128:(c2 + 1) * 128],
                        xts[dc][:, lo:hi],
                        start=(dc == 0),
                        stop=(dc == 7),
                    )
                # eviction cast + bias add in one DVE pass (bias is
                # per-partition in the q^T layout)
                raw = rawpool.tile([128, 512], BF16, tag="raw")
                nc.vector.tensor_scalar_add(
                    raw[:], ps[:], bcol_sb[:, 2 * ti + c2:2 * ti + c2 + 1])
                # RoPE: fin = raw*cos + rot(raw)*sin_signed
                f = fin
                for (do, di) in ((0, 32), (32, 0), (64, 96), (96, 64)):
                    nc.vector.tensor_copy(f[do:do + 32, lo:hi], raw[di:di + 32, :])
                nc.vector.tensor_mul(f[:, lo:hi], f[:, lo:hi], sin_sb[:, lo:hi])
                nc.vector.tensor_mul(raw[:], raw[:], cos_sb[:, lo:hi])
                # final add on Pool (idle) to unload DVE
                nc.gpsimd.tensor_add(f[:, lo:hi], f[:, lo:hi], raw[:])

            # ---- V projection (bf16, bias as rank-1, ones denominator cols)
            vts = [None] * NK

            def proj_v(kt):
                ps = psP.tile([128, HD], F32, tag="pp")
                for dc in range(8):
                    nc.tensor.matmul(
                        ps[:],
                        xts[dc][:, kt * 128:(kt + 1) * 128],
                        wv_sb[:, dc, :],
                        start=(dc == 0),
                        stop=False,
                    )
                nc.tensor.matmul(
                    ps[:], ones_sb[0:1, :], bvrow_sb[0:1, :],
                    start=False, stop=True,
                )
                vt = xwpool.tile([128, HG, 128], BF16, tag=f"v{kt}", name=f"v{kt}")
                nc.vector.tensor_copy(
                    vt[:, :, 0:64],
                    ps[:].rearrange("p (h d) -> p h d", h=HG),
                )
                nc.vector.memset(vt[:, :, 64:128], 1.0)
                vts[kt] = vt

            # ---- attention for head pair c2, one q-quarter (512 cols) ----
            # scores/probs/Y for both heads side by side in one [128, 2, 512]
            # tile.  Y matmuls run TWO key-tiles behind their scores so they
            # never wait on exp; fillers (independent PE units) are popped
            # one per two kt steps to keep the PE dense without starving the
            # exp stream of scores.
            def attn_quarter(c2, qq, fillers=()):
                fillers = list(fillers)
                qlo = qq * 512
                last = 4 * qq + 3
                y = psY.tile([128, 2, 512], F32, tag="y", name=f"y{c2}_{qq}")
                pend = []   # [(kt, c, p)] emitted scores/exp awaiting Y

                def emit_y():
                    kt, c, p = pend.pop(0)
                    for h in (0, 1):
                        nc.tensor.matmul(
                            y[:, h, c:],
                            vts[kt][:, 2 * c2 + h, :],
                            p[:, h, c:],
                            start=(kt == 0),
                            stop=(kt == last),
                            skip_group_check=True,
                        )

                for kt in range(last + 1):
                    j = kt - 4 * qq
                    c = j * 128 if j >= 0 else 0   # first valid col (diag trim)
                    ksl = slice(kt * 128, (kt + 1) * 128)
                    qsl = slice(qlo + c, qlo + 512)
                    s = psS.tile([128, 2, 512], F32, tag="s")
                    # two row-group-concurrent 64-row score matmuls
                    nc.tensor.matmul(
                        s[:, 0, c:], kT[c2][0:64, ksl], qT[c2][0:64, qsl],
                        start=True, stop=True,
                    )
                    nc.tensor.matmul(
                        s[:, 1, c:], kT[c2][64:128, ksl], qT[c2][64:128, qsl],
                        start=True, stop=True,
                    )
                    if j >= 0:
                        # diagonal block: add NEGM strict upper triangle for
                        # both heads (exp then underflows to 0)
                        nc.tensor.matmul(
                            s[:, :, c:c + 128], maskt_sb[:], id2_sb[:],
                            start=False, stop=True, skip_group_check=True,
                        )
                    p = ppool.tile([128, 2, 512], BF16, tag="p")
                    nc.scalar.activation(
                        p[:, :, c:], s[:, :, c:],
                        mybir.ActivationFunctionType.Exp,
                        bias=kb_sb[:, kt:kt + 1], scale=SCALE,
                    )
                    pend.append((kt, c, p))
                    if len(pend) > 2:
                        emit_y()
                    if fillers and kt % 2 == 1:
                        fillers.pop(0)()
                while pend:
                    emit_y()
                # normalize both heads at once: 1/r = exp(-ln r) (ln and exp
                # share an ACT table so no table reload)
                lnr = rpool.tile([64, 2, 512], F32, tag="lnr")
                rec = rpool.tile([64, 2, 512], F32, tag="rec")
                nc.scalar.activation(
                    lnr[:], y[64:128, :, :], mybir.ActivationFunctionType.Ln)
                nc.scalar.activation(
                    rec[:], lnr[:], mybir.ActivationFunctionType.Exp,
                    scale=-1.0)
                nc.vector.tensor_mul(
                    yT[c2][0:64, qlo:qlo + 512], y[0:64, 0, :], rec[:, 0, :])
                nc.vector.tensor_mul(
                    yT[c2][64:128, qlo:qlo + 512], y[0:64, 1, :], rec[:, 1, :])
                for fill in fillers:
                    fill()

            def outproj_tt(tt, on_scalar=False):
                pss = [psP.tile([128, 512], F32, tag="pp", name=f"po{t}")
                       for t in range(2)]
                for c2 in range(2):
                    for dr in range(2):
                        nc.tensor.matmul(
                            pss[dr][:],
                            yT[c2][:, tt * 128:(tt + 1) * 128],
                            wo_sb[:, c2, dr * 512:(dr + 1) * 512],
                            start=(c2 == 0),
                            stop=(c2 == 1),
                            skip_group_check=True,
                        )
                for dr in range(2):
                    ev = evpool.tile([128, 512], BF16, tag="ev")
                    if on_scalar:
                        nc.scalar.activation(
                            ev[:], pss[dr][:],
                            mybir.ActivationFunctionType.Identity)
                    else:
                        nc.vector.tensor_copy(ev[:], pss[dr][:])
                    nc.sync.dma_start(
                        out[tt * 128:(tt + 1) * 128, dr * 512:(dr + 1) * 512],
                        ev[:],
                    )

            def F(fn, *a, **k):
                return lambda: fn(*a, **k)

            # ---- emission order == scheduler priority ----
            # quarter qq of pair c2 needs q/k chunks tr<=qq (of pair c2) and
            # V tiles kt<=4qq+3.  Fillers are placed so each unit's inputs
            # are ready one quarter ahead of use.
            proj_qk_chunk(0, wq_sb, 0, qT[0], 0)
            proj_qk_chunk(1, wk_sb, 0, kT[0], 0)
            for kt in range(0, 4):
                proj_v(kt)
            attn_quarter(0, 0, [
                F(proj_qk_chunk, 0, wq_sb, 0, qT[0], 1),
                F(proj_qk_chunk, 1, wk_sb, 0, kT[0], 1),
            ])
            for kt in range(4, 8):
                proj_v(kt)
            attn_quarter(0, 1, [
                F(proj_qk_chunk, 0, wq_sb, 0, qT[0], 2),
                F(proj_qk_chunk, 1, wk_sb, 0, kT[0], 2),
                F(proj_v, 8),
                F(proj_v, 9),
            ])
            attn_quarter(0, 2, [
                F(proj_qk_chunk, 0, wq_sb, 0, qT[0], 3),
                F(proj_qk_chunk, 1, wk_sb, 0, kT[0], 3),
                F(proj_v, 10),
                F(proj_v, 11),
                F(proj_v, 12),
                F(proj_v, 13),
            ])
            attn_quarter(0, 3, [
                F(proj_v, 14),
                F(proj_v, 15),
                F(proj_qk_chunk, 0, wq_sb, 1, qT[1], 0),
                F(proj_qk_chunk, 1, wk_sb, 1, kT[1], 0),
                F(proj_qk_chunk, 0, wq_sb, 1, qT[1], 1),
                F(proj_qk_chunk, 1, wk_sb, 1, kT[1], 1),
                F(proj_qk_chunk, 0, wq_sb, 1, qT[1], 2),
                F(proj_qk_chunk, 1, wk_sb, 1, kT[1], 2),
            ])
            attn_quarter(1, 0, [
                F(proj_qk_chunk, 0, wq_sb, 1, qT[1], 3),
                F(proj_qk_chunk, 1, wk_sb, 1, kT[1], 3),
            ])
            attn_quarter(1, 1, [
                F(outproj_tt, 0),
                F(outproj_tt, 1),
                F(outproj_tt, 2),
                F(outproj_tt, 3),
            ])
            attn_quarter(1, 2, [
                F(outproj_tt, 4),
                F(outproj_tt, 5),
                F(outproj_tt, 6),
                F(outproj_tt, 7),
            ])
            attn_quarter(1, 3, [
                F(outproj_tt, 8),
                F(outproj_tt, 9),
                F(outproj_tt, 10),
                F(outproj_tt, 11),
            ])
            for tt in range(12, 16):
                outproj_tt(tt, on_scalar=True)
    nd = _dedup_ldweights(nc)
    _split_multi_waits(nc)
    assert nd > 0, f"expected ldweights dedup to fire, got {nd}"
    return nc


def _rope_tables():
    inv_freq = 1.0 / (THETA ** (np.arange(0, HS, 2, dtype=np.float64) / HS))  # [32]
    t = np.arange(T, dtype=np.float64)
    fr = t[:, None] * inv_freq[None, :]          # [T, 32]
    emb = np.concatenate([fr, fr], axis=1)       # [T, 64]
    cos = np.cos(emb).T.astype(np.float32)       # [64, T]
    sin = np.sin(emb).T.astype(np.float32)       # [64, T]
    sin_signed = sin.copy()
    sin_signed[0:32] = -sin_signed[0:32]
    cos2 = np.concatenate([cos, cos], axis=0)            # [128, T]
    sin2 = np.concatenate([sin_signed, sin_signed], 0)   # [128, T]
    return cos2.astype(ml_dtypes.bfloat16), sin2.astype(ml_dtypes.bfloat16)


def _in_maps(x, attention_mask, Wq, bqv, Wk, bkv, Wv, bvv, Wo):
    cos2, sin2 = _rope_tables()
    maskt = (NEGM * np.triu(np.ones((128, 128), np.float32), k=1)
             ).astype(ml_dtypes.bfloat16)
    id2 = np.concatenate([np.eye(128, dtype=np.float32)] * 2, axis=1
                         ).astype(ml_dtypes.bfloat16)
    bf = ml_dtypes.bfloat16
    xTs = [np.ascontiguousarray(x[b].T).astype(bf) for b in range(B)]
    kbs = [
        np.where(attention_mask[b] != 0, 0.0, NEG).astype(np.float32)
        for b in range(B)
    ]
    maps = []
    for core in range(NCORES):
        b, g = core // 4, core % 4
        sl = slice(g * HD, (g + 1) * HD)
        # bias columns [128, 4]: [:, 2*ti+c2]
        bcol = np.stack([
            bqv[sl][0:128], bqv[sl][128:256],
            bkv[sl][0:128], bkv[sl][128:256],
        ], axis=1).astype(np.float32)
        maps.append({
            "xT": xTs[b],
            "wq": np.ascontiguousarray(Wq[:, sl]).astype(bf),
            "wk": np.ascontiguousarray(Wk[:, sl]).astype(bf),
            "wv": np.ascontiguousarray(Wv[:, sl]).astype(bf),
            "wo": np.ascontiguousarray(Wo[sl, :]).astype(bf),
            "bcol": bcol,
            "bvrow": bvv[sl].astype(bf).reshape(1, HD),
            "cos2": cos2,
            "sin2": sin2,
            "maskt": maskt,
            "id2": id2,
            "kb": kbs[b],
        })
    return maps


def _run(inputs, trace=False):
    global _NC
    if _NC is None:
        _NC = build_nc()
    maps = _in_maps(
        np.asarray(inputs["x"]), np.asarray(inputs["attention_mask"]),
        np.asarray(inputs["Wq"]), np.asarray(inputs["bq"]),
        np.asarray(inputs["Wk"]), np.asarray(inputs["bk"]),
        np.asarray(inputs["Wv"]), np.asarray(inputs["bv"]),
        np.asarray(inputs["Wo"]),
    )
    res = run_bass_kernel_spmd(_NC, maps, core_ids=list(range(NCORES)), trace=trace)
    bo = np.asarray(inputs["bo"], np.float32)
    outs = []
    for b in range(B):
        acc = np.zeros((T, D), np.float32)
        for g in range(4):
            acc += np.asarray(res.results[b * 4 + g]["out"], np.float32)
        outs.append(acc + bo[None, :])
    return np.stack(outs, axis=0), res


def kernel(**inputs):
    out, _ = _run(inputs, trace=False)
    return out


# revision 10
# speedup vs baseline: 1.2582x; 1.0338x over previous
"""Distributed Trainium2 Bass kernel for a 16-head causal RoPE attention layer.

Problem: B=2, T=2048, D=1024, H=16, HS=64 (fp32 reference).

Sharding (8 cores): core = b*4 + g, b in {0,1} (batch), g in {0..3} (group of
4 heads).  Each core computes Q/K/V projections for its 256 head-dims, runs
causal attention for its 4 heads, and applies its 256-row slice of Wo,
producing a partial [T, D] output.  The host sums the 4 partials per batch
and adds bo.  No on-device collectives.

v5: all-bf16 compute (fp8 q/k measured 1.7e-2 rel err from softmax weight
noise -- too close to the 2e-2 gate), with the time recovered by keeping
the PE dense (Trainium2's PE downclocks 2.4->1.2GHz after any pipeline
gap and needs ~3us of continuous work to ramp back):
  - attention Y matmuls are software-pipelined TWO key-tiles behind their
    scores, so Y(kt-2) never waits on exp(kt-2) (it finished during the
    scores of kt-1/kt) and the PE stream has no per-kt dependency stall.
  - small independent PE units (projection chunks, V tiles, outproj
    T-tiles) are scattered one-per-two-kt-steps inside the attention
    quarters as fillers, sized so the per-kt PE time tracks the ~1.2us
    exp op and the ACT stream is never starved of scores.
  - q/k biases ride the PSUM eviction as a DVE tensor_scalar add (bias is
    per-partition in the q^T layout) -- no rank-1 bias matmuls.
  - causal masking: the diagonal 128x128 strict upper triangle is added
    as a -4e5 stationary matmul into the scores psum (PE), so exp
    underflows to 0 and DVE never touches the exp->Y path.
  - V bias via rank-1 matmul; normalize via ln/exp on ACT (shared table).
ACT is the bottleneck (~90us exp + ~21us ln/exp normalize); PE (~105us)
runs just under it when dense, DVE ~85us, Pool ~23us (RoPE adds).
PSUM: scores 2x2 banks + Y 2 banks + proj/outproj ring 2x1 = 8 exactly.
"""

import numpy as np
import ml_dtypes

import concourse.bass as bass
import concourse.mybir as mybir
import concourse.tile as tile
from concourse.bass_utils import run_bass_kernel_spmd

BF16 = mybir.dt.bfloat16
F32 = mybir.dt.float32

B, T, D = 2, 2048, 1024
H, HS = 16, 64
THETA = 10000.0
NCORES = 8
HG = 4            # heads per core
HD = HG * HS      # head dims per core = 256
SCALE = 1.0 / 8.0  # 1/sqrt(HS)
NEG = -1.0e5       # additive mask for padded keys (exp underflows to 0)
NEGM = -4.0e5      # diagonal-mask matmul constant (survives bf16 p exactly as 0)

_NC = None


_SELF_SEM = {
    "EngineType.Activation": "Activation_",
    "EngineType.DVE": "DVE_",
    "EngineType.PE": "PE_",
    "EngineType.Pool": "Pool_",
}


def _split_multi_waits(nc):
    """walrus codegen accepts at most ONE semaphore wait per engine
    instruction (the 64B ISA structs have a single EVENTS slot); Tile's
    scheduler freely emits several.  Hoist all but the last wait of each
    instruction onto inserted same-engine EventSemaphore (poll_sem) ops,
    which preserves semantics exactly (engines execute sequentially).

    Additionally drop ge-waits on the instruction's OWN engine semaphore
    for compute engines: those guard WAW/WAR against earlier instructions
    of the same in-order engine, which program order already guarantees."""
    def _names(args):
        out = set()
        for a in args:
            for attr in ("memref", "name"):
                v = getattr(a, attr, None)
                if isinstance(v, str):
                    out.add(v.removesuffix("_set"))
            t = getattr(a, "tensor", None)
            if t is not None and isinstance(getattr(t, "name", None), str):
                out.add(t.name)
        return out

    eng_written = {}
    eng_read = {}
    _COMPUTE = {"InstActivation", "InstTensorTensor", "InstTensorCopy",
                "InstMatmult", "InstLdweights", "InstMemset",
                "InstTensorScalarPtr", "InstTensorReduce"}
    for f in nc.m.functions:
        for blk in f.blocks:
            for inst in blk.instructions:
                if type(inst).__name__ in _COMPUTE:
                    e = str(inst.engine)
                    eng_written.setdefault(e, set()).update(_names(inst.outs))
                    eng_read.setdefault(e, set()).update(_names(inst.ins))

    n = 0
    for f in nc.m.functions:
        for blk in f.blocks:
            il = blk.instructions
            i = 0
            while i < len(il):
                inst = il[i]
                si = inst.sync_info
                if si is None or not si.on_wait:
                    i += 1
                    continue
                waits = list(si.on_wait)
                eng = str(inst.engine)
                selfpfx = _SELF_SEM.get(eng)
                if (selfpfx is not None
                        and type(inst).__name__ in (
                            "InstActivation", "InstMatmult", "InstLdweights",
                            "InstTensorTensor", "InstTensorCopy", "InstMemset")
                        and not (_names(inst.ins) & eng_written.get(eng, set()))
                        and not (_names(inst.outs) & eng_read.get(eng, set()))):
                    kept = [w for w in waits
                            if not (w.wait_mode == "sem-ge-imm"
                                    and w.ant_name.startswith(selfpfx))]
                    if len(kept) != len(waits):
                        waits = kept
                        inst.sync_info = mybir.SyncInfo(
                            on_wait=waits, on_update=list(si.on_update))
                if len(waits) > 1:
                    for w in waits[:-1]:
                        es = mybir.InstEventSemaphore(name=f"I-wsplit-{n}")
                        n += 1
                        es.engine = inst.engine
                        es.sync_info = mybir.SyncInfo(on_wait=[w], on_update=[])
                        nc.register_instruction(es)
                        il.insert(i, es)
                        i += 1
                    inst.sync_info = mybir.SyncInfo(
                        on_wait=[waits[-1]], on_update=list(si.on_update))
                i += 1
    return n


def _dedup_ldweights(nc):
    """bass emits one InstLdweights per InstMatmult.  When a later
    InstLdweights loads the IDENTICAL weights AP that is already resident
    in the PE array (no other InstLdweights in between), the reload is
    redundant: MATMUL does not self-load for 16-bit dtypes.  Delete it,
    folding its waits into the following matmult."""
    def fp(inst):
        a = inst.ins[0]
        return (a.memref, a.offset, str(a.ap), str(a.dtype),
                str(getattr(inst, "perf_mode", None)))

    n = 0
    for f in nc.m.functions:
        for blk in f.blocks:
            il = blk.instructions
            last = None
            i = 0
            while i < len(il):
                inst = il[i]
                tn = type(inst).__name__
                if tn == "InstLdweights":
                    cur = fp(inst)
                    si = inst.sync_info
                    if cur == last and (si is None or not si.on_update):
                        waits = list(si.on_wait) if si is not None else []
                        if waits:
                            j = i + 1
                            while (j < len(il)
                                   and type(il[j]).__name__ != "InstMatmult"):
                                j += 1
                            if j == len(il):
                                i += 1
                                continue
                            mm = il[j]
                            msi = mm.sync_info
                            mw = list(msi.on_wait) if msi is not None else []
                            mu = list(msi.on_update) if msi is not None else []
                            mm.sync_info = mybir.SyncInfo(
                                on_wait=waits + mw, on_update=mu)
                        del il[i]
                        n += 1
                        continue
                    last = cur
                i += 1
    return n


def build_nc():
    nc = bass.Bass()

    xT = nc.declare_dram_parameter("xT", [D, T], BF16, isOutput=False)
    wq = nc.declare_dram_parameter("wq", [D, HD], BF16, isOutput=False)
    wk = nc.declare_dram_parameter("wk", [D, HD], BF16, isOutput=False)
    wv = nc.declare_dram_parameter("wv", [D, HD], BF16, isOutput=False)
    wo = nc.declare_dram_parameter("wo", [HD, D], BF16, isOutput=False)
    # per-partition bias columns: [:, 2*ti+c2] = bias for q/k (ti) pair c2
    bcol = nc.declare_dram_parameter("bcol", [128, 4], F32, isOutput=False)
    bvrow = nc.declare_dram_parameter("bvrow", [1, HD], BF16, isOutput=False)
    cos2 = nc.declare_dram_parameter("cos2", [128, T], BF16, isOutput=False)
    sin2 = nc.declare_dram_parameter("sin2", [128, T], BF16, isOutput=False)
    # NEGM * strict upper triangle (stationary for the diag mask matmul)
    maskt = nc.declare_dram_parameter("maskt", [128, 128], BF16, isOutput=False)
    # identity repeated twice (moving for the diag mask matmul)
    id2 = nc.declare_dram_parameter("id2", [128, 256], BF16, isOutput=False)
    kb = nc.declare_dram_parameter("kb", [T], F32, isOutput=False)
    out = nc.declare_dram_parameter("out", [T, D], BF16, isOutput=True)

    NK = T // 128   # 16 key tiles

    with tile.TileContext(nc) as tc:
        with (
            tc.tile_pool(name="const", bufs=1) as cpool,
            tc.tile_pool(name="xw", bufs=1) as xwpool,
            tc.tile_pool(name="qk", bufs=1) as qkpool,
            tc.tile_pool(name="raw", bufs=3) as rawpool,
            tc.tile_pool(name="p", bufs=6) as ppool,
            tc.tile_pool(name="rec", bufs=2) as rpool,
            tc.tile_pool(name="yr", bufs=2) as yrpool,
            tc.tile_pool(name="ev", bufs=3) as evpool,
            tc.tile_pool(name="psP", bufs=2, space="PSUM") as psP,
            tc.tile_pool(name="psS", bufs=2, space="PSUM") as psS,
            tc.tile_pool(name="psY", bufs=1, space="PSUM") as psY,
        ):
            # ---- constant / weight loads ----
            # wq + the first xT column-chunks gate the first matmul groups;
            # xT is loaded in 512-col chunks so projections start early.
            wq_sb = xwpool.tile([128, 8, HD], BF16, tag="wq")
            wk_sb = xwpool.tile([128, 8, HD], BF16, tag="wk")
            wv_sb = xwpool.tile([128, 8, HD], BF16, tag="wv")
            wo_sb = xwpool.tile([128, 2, D], BF16, tag="wo")
            bcol_sb = cpool.tile([128, 4], F32, tag="bcol")
            bvrow_sb = cpool.tile([1, HD], BF16, tag="bvrow")
            ones_sb = cpool.tile([1, 128], BF16, tag="ones")
            wq_r = wq.ap().rearrange("(c p) n -> p c n", p=128)
            wk_r = wk.ap().rearrange("(c p) n -> p c n", p=128)
            # pair-0 (c2=0) weight halves first: the first attention quarter
            # needs only these
            nc.sync.dma_start(wq_sb[:, :, 0:128], wq_r[:, :, 0:128])
            nc.sync.dma_start(bcol_sb[:], bcol.ap())
            nc.sync.dma_start(bvrow_sb[:], bvrow.ap())
            nc.vector.memset(ones_sb[:], 1.0)

            xts = []
            for dc in range(8):
                xt = xwpool.tile([128, T], BF16, tag=f"xt{dc}", name=f"xt{dc}")
                xts.append(xt)

            def load_xt_tr(tr):
                for dc in range(8):
                    nc.sync.dma_start(
                        xts[dc][:, tr * 512:(tr + 1) * 512],
                        xT[dc * 128:(dc + 1) * 128, tr * 512:(tr + 1) * 512],
                    )

            cos_sb = cpool.tile([128, T], BF16, tag="cos")
            sin_sb = cpool.tile([128, T], BF16, tag="sin")
            maskt_sb = cpool.tile([128, 128], BF16, tag="maskt")
            id2_sb = cpool.tile([128, 2, 128], BF16, tag="id2")
            kb_sb = cpool.tile([128, NK], F32, tag="kb")
            load_xt_tr(0)
            nc.sync.dma_start(wk_sb[:, :, 0:128], wk_r[:, :, 0:128])
            nc.sync.dma_start(cos_sb[:, 0:512], cos2[:, 0:512])
            nc.sync.dma_start(sin_sb[:, 0:512], sin2[:, 0:512])
            nc.sync.dma_start(wv_sb[:], wv.ap().rearrange("(c p) n -> p c n", p=128))
            nc.sync.dma_start(maskt_sb[:], maskt.ap())
            nc.sync.dma_start(
                id2_sb[:], id2.ap().rearrange("p (two q) -> p two q", two=2))
            nc.sync.dma_start(kb_sb[:], kb.ap().rearrange("(t p) -> p t", p=128))
            load_xt_tr(1)
            nc.sync.dma_start(cos_sb[:, 512:T], cos2[:, 512:T])
            nc.sync.dma_start(sin_sb[:, 512:T], sin2[:, 512:T])
            load_xt_tr(2)
            load_xt_tr(3)
            nc.sync.dma_start(wq_sb[:, :, 128:256], wq_r[:, :, 128:256])
            nc.sync.dma_start(wk_sb[:, :, 128:256], wk_r[:, :, 128:256])
            nc.sync.dma_start(wo_sb[:], wo.ap().rearrange("(c p) n -> p c n", p=128))

            # persistent [128, T] tiles: 2 heads each (rows 0:64 / 64:128)
            qT = [qkpool.tile([128, T], BF16, tag=f"qT{c}", name=f"qT{c}") for c in range(2)]
            kT = [qkpool.tile([128, T], BF16, tag=f"kT{c}", name=f"kT{c}") for c in range(2)]
            yT = [qkpool.tile([128, T], BF16, tag=f"yT{c}", name=f"yT{c}") for c in range(2)]

            # ---- Q^T / K^T projection + RoPE, one 512-col chunk ----
            # ti: 0 = q, 1 = k (selects bias column)
            def proj_qk_chunk(ti, wsb, c2, fin, tr):
                lo, hi = tr * 512, (tr + 1) * 512
                ps = psP.tile([128, 512], F32, tag="pp")
                for dc in range(8):
                    nc.tensor.matmul(
                        ps[:],
                        wsb[:, dc, c2 * 128:(c2 + 1) * 128],
                        xts[dc][:, lo:hi],
                        start=(dc == 0),
                        stop=(dc == 7),
                    )
                # eviction cast + bias add in one DVE pass (bias is
                # per-partition in the q^T layout)
                raw = rawpool.tile([128, 512], BF16, tag="raw")
                nc.vector.tensor_scalar_add(
                    raw[:], ps[:], bcol_sb[:, 2 * ti + c2:2 * ti + c2 + 1])
                # RoPE: fin = raw*cos + rot(raw)*sin_signed
                f = fin
                for (do, di) in ((0, 32), (32, 0), (64, 96), (96, 64)):
                    nc.vector.tensor_copy(f[do:do + 32, lo:hi], raw[di:di + 32, :])
                nc.vector.tensor_mul(f[:, lo:hi], f[:, lo:hi], sin_sb[:, lo:hi])
                nc.vector.tensor_mul(raw[:], raw[:], cos_sb[:, lo:hi])
                # final add on Pool (idle) to unload DVE
                nc.gpsimd.tensor_add(f[:, lo:hi], f[:, lo:hi], raw[:])

            # ---- V projection (bf16, bias as rank-1, ones denominator cols)
            vts = [None] * NK

            def proj_v(kt):
                ps = psP.tile([128, HD], F32, tag="pp")
                for dc in range(8):
                    nc.tensor.matmul(
                        ps[:],
                        xts[dc][:, kt * 128:(kt + 1) * 128],
                        wv_sb[:, dc, :],
                        start=(dc == 0),
                        stop=False,
                    )
                nc.tensor.matmul(
                    ps[:], ones_sb[0:1, :], bvrow_sb[0:1, :],
                    start=False, stop=True,
                )
                vt = xwpool.tile([128, HG, 128], BF16, tag=f"v{kt}", name=f"v{kt}")
                nc.vector.tensor_copy(
                    vt[:, :, 0:64],
                    ps[:].rearrange("p (h d) -> p h d", h=HG),
                )
                nc.vector.memset(vt[:, :, 64:128], 1.0)
                vts[kt] = vt

            # ---- attention: one flat driver over (pair, quarter, kt) ----
            # scores/probs/Y for both heads side by side in one [128, 2, 512]
            # tile.  Y matmuls run TWO key-tiles behind their scores (so they
            # never wait on exp), and the pipeline runs ACROSS quarter
            # boundaries: a quarter's trailing Y matmuls interleave with the
            # next quarter's scores.  After the stop-Y, the y psum is evicted
            # raw (f32) to SBUF on DVE, freeing the single psY buffer ~1.1us
            # later; the ln/exp normalize reads the SBUF copy and is emitted
            # under the NEXT quarter's exp stream so ACT never waits for it.
            # Fillers (independent PE units, 0.4-1us) are popped one per kt.
            pend = []     # [(y, c2, kt, c, p, start, stop, fin)]

            def emit_y():
                y, c2, kt, c, p, st, sp, fin = pend.pop(0)
                for h in (0, 1):
                    nc.tensor.matmul(
                        y[:, h, c:],
                        vts[kt][:, 2 * c2 + h, :],
                        p[:, h, c:],
                        start=st,
                        stop=sp,
                        skip_group_check=True,
                    )
                if sp:
                    fin()

            def attn_quarter(c2, qq, fillers=(), norm_prev=None, tail=False):
                fillers = list(fillers)
                qlo = qq * 512
                last = 4 * qq + 3
                y = psY.tile([128, 2, 512], F32, tag="y", name=f"y{c2}_{qq}")

                def fin():
                    # raw eviction (numerators + denominators) to SBUF f32;
                    # the normalize is emitted later, under the next
                    # quarter's exp stream.
                    yraw = yrpool.tile([128, 2, 512], F32, tag="yr")
                    nc.vector.tensor_copy(yraw[:], y[:])
                    norm_q.append((c2, qq, yraw))

                for kt in range(last + 1):
                    j = kt - 4 * qq
                    c = j * 128 if j >= 0 else 0   # first valid col (diag trim)
                    ksl = slice(kt * 128, (kt + 1) * 128)
                    qsl = slice(qlo + c, qlo + 512)
                    s = psS.tile([128, 2, 512], F32, tag="s")
                    # two row-group-concurrent 64-row score matmuls
                    nc.tensor.matmul(
                        s[:, 0, c:], kT[c2][0:64, ksl], qT[c2][0:64, qsl],
                        start=True, stop=True,
                    )
                    nc.tensor.matmul(
                        s[:, 1, c:], kT[c2][64:128, ksl], qT[c2][64:128, qsl],
                        start=True, stop=True,
                    )
                    if j >= 0:
                        # diagonal block: add NEGM strict upper triangle for
                        # both heads (exp then underflows to 0)
                        nc.tensor.matmul(
                            s[:, :, c:c + 128], maskt_sb[:], id2_sb[:],
                            start=False, stop=True, skip_group_check=True,
                        )
                    p = ppool.tile([128, 2, 512], BF16, tag="p")
                    nc.scalar.activation(
                        p[:, :, c:], s[:, :, c:],
                        mybir.ActivationFunctionType.Exp,
                        bias=kb_sb[:, kt:kt + 1], scale=SCALE,
                    )
                    if kt == 2 and norm_prev is not None:
                        # the previous quarter's stop-Y (and its raw
                        # eviction) was emitted at kt1; its normalize rides
                        # here, under this quarter's exp stream
                        norm_prev()
                    pend.append((y, c2, kt, c, p, kt == 0, kt == last, fin))
                    if len(pend) > 2:
                        emit_y()
                    if fillers:
                        f = fillers.pop(0)
                        if f is not None:
                            f()
                if tail:
                    while pend:
                        emit_y()
                    emit_norm()

            norm_q = []   # completed quarters awaiting normalize

            def emit_norm():
                c2, qq, yraw = norm_q.pop(0)
                qlo = qq * 512
                # 1/r = exp(-ln r): ln and exp share an ACT table (no reload)
                lnr = rpool.tile([64, 2, 512], F32, tag="lnr")
                rec = rpool.tile([64, 2, 512], F32, tag="rec")
                nc.scalar.activation(
                    lnr[:], yraw[64:128, :, :],
                    mybir.ActivationFunctionType.Ln)
                nc.scalar.activation(
                    rec[:], lnr[:], mybir.ActivationFunctionType.Exp,
                    scale=-1.0)
                nc.vector.tensor_mul(
                    yT[c2][0:64, qlo:qlo + 512], yraw[0:64, 0, :], rec[:, 0, :])
                nc.vector.tensor_mul(
                    yT[c2][64:128, qlo:qlo + 512], yraw[0:64, 1, :], rec[:, 1, :])

            # outproj split in two halves (one per head pair), each using one
            # psum tile per wo column half; the yT stationary slice serves
            # both column halves back-to-back (dedup deletes the reload).
            op_state = {}

            def outproj_half(tt, c2, on_scalar=False):
                if c2 == 0:
                    op_state[tt] = [
                        psP.tile([128, 512], F32, tag="pp", name=f"po{t}")
                        for t in range(2)]
                pss = op_state[tt]
                for dr in range(2):
                    nc.tensor.matmul(
                        pss[dr][:],
                        yT[c2][:, tt * 128:(tt + 1) * 128],
                        wo_sb[:, c2, dr * 512:(dr + 1) * 512],
                        start=(c2 == 0),
                        stop=(c2 == 1),
                        skip_group_check=True,
                    )
                if c2 == 1:
                    for dr in range(2):
                        ev = evpool.tile([128, 512], BF16, tag="ev")
                        if on_scalar:
                            nc.scalar.activation(
                                ev[:], pss[dr][:],
                                mybir.ActivationFunctionType.Identity)
                        else:
                            nc.vector.tensor_copy(ev[:], pss[dr][:])
                        nc.sync.dma_start(
                            out[tt * 128:(tt + 1) * 128,
                                dr * 512:(dr + 1) * 512],
                            ev[:],
                        )

            # projection chunks split in two 4-matmul halves for finer
            # filler placement; the psum tile is shared via pr_state.
            pr_state = {}

            def proj_half(ti, wsb, c2, fin, tr, second):
                key = (ti, c2, tr)
                lo, hi = tr * 512, (tr + 1) * 512
                if not second:
                    pr_state[key] = psP.tile([128, 512], F32, tag="pp",
                                             name="prh")
                ps = pr_state[key]
                for dc in (range(4, 8) if second else range(4)):
                    nc.tensor.matmul(
                        ps[:],
                        wsb[:, dc, c2 * 128:(c2 + 1) * 128],
                        xts[dc][:, lo:hi],
                        start=(dc == 0),
                        stop=(dc == 7),
                    )
                if second:
                    proj_rope(ti, c2, fin, tr, ps)

            def proj_rope(ti, c2, fin, tr, ps):
                lo, hi = tr * 512, (tr + 1) * 512
                raw = rawpool.tile([128, 512], BF16, tag="raw")
                nc.vector.tensor_scalar_add(
                    raw[:], ps[:], bcol_sb[:, 2 * ti + c2:2 * ti + c2 + 1])
                f = fin
                for (do, di) in ((0, 32), (32, 0), (64, 96), (96, 64)):
                    nc.vector.tensor_copy(f[do:do + 32, lo:hi], raw[di:di + 32, :])
                nc.vector.tensor_mul(f[:, lo:hi], f[:, lo:hi], sin_sb[:, lo:hi])
                nc.vector.tensor_mul(raw[:], raw[:], cos_sb[:, lo:hi])
                nc.gpsimd.tensor_add(f[:, lo:hi], f[:, lo:hi], raw[:])

            def F(fn, *a, **k):
                return lambda: fn(*a, **k)

            def PJ(ti, wsb, c2, fin, tr):
                return [F(proj_half, ti, wsb, c2, fin, tr, False),
                        F(proj_half, ti, wsb, c2, fin, tr, True)]

            def OP(tt, on_scalar=False):
                return [F(outproj_half, tt, 0),
                        F(outproj_half, tt, 1, on_scalar)]

            # ---- emission order == scheduler priority ----
            # quarter qq of pair c2 needs q/k chunks tr<=qq (of pair c2) and
            # V tiles kt<=4qq+3; each filler unit's inputs are ready at
            # least two kt steps ahead of first use.
            proj_qk_chunk(0, wq_sb, 0, qT[0], 0)
            proj_qk_chunk(1, wk_sb, 0, kT[0], 0)
            for kt in range(0, 4):
                proj_v(kt)
            attn_quarter(0, 0,
                         PJ(0, wq_sb, 0, qT[0], 1) + PJ(1, wk_sb, 0, kT[0], 1))
            attn_quarter(0, 1,
                         [F(proj_v, 4), F(proj_v, 5)]
                         + PJ(0, wq_sb, 0, qT[0], 2)
                         + [F(proj_v, 6), F(proj_v, 7)]
                         + PJ(1, wk_sb, 0, kT[0], 2),
                         norm_prev=emit_norm)
            attn_quarter(0, 2,
                         [F(proj_v, 8), F(proj_v, 9)]
                         + PJ(0, wq_sb, 0, qT[0], 3)
                         + [F(proj_v, 10), F(proj_v, 11)]
                         + PJ(1, wk_sb, 0, kT[0], 3)
                         + [F(proj_v, 12), F(proj_v, 13)],
                         norm_prev=emit_norm)
            attn_quarter(0, 3,
                         [F(proj_v, 14), F(proj_v, 15)]
                         + PJ(0, wq_sb, 1, qT[1], 0)
                         + PJ(1, wk_sb, 1, kT[1], 0)
                         + PJ(0, wq_sb, 1, qT[1], 1)
                         + PJ(1, wk_sb, 1, kT[1], 1)
                         + PJ(0, wq_sb, 1, qT[1], 2),
                         norm_prev=emit_norm)
            attn_quarter(1, 0,
                         PJ(1, wk_sb, 1, kT[1], 2),
                         norm_prev=emit_norm)
            attn_quarter(1, 1,
                         PJ(0, wq_sb, 1, qT[1], 3)
                         + PJ(1, wk_sb, 1, kT[1], 3)
                         + OP(0) + OP(1),
                         norm_prev=emit_norm)
            attn_quarter(1, 2,
                         [None, None]
                         + OP(2) + OP(3) + OP(4) + OP(5) + OP(6),
                         norm_prev=emit_norm)
            attn_quarter(1, 3,
                         [None, None]
                         + OP(7) + OP(8) + OP(9) + OP(10) + OP(11),
                         norm_prev=emit_norm, tail=True)
            for tt in range(12, 16):
                for f in OP(tt, on_scalar=True):
                    f()
    nd = _dedup_ldweights(nc)
    _split_multi_waits(nc)
    assert nd > 0, f"expected ldweights dedup to fire, got {nd}"
    return nc


def _rope_tables():
    inv_freq = 1.0 / (THETA ** (np.arange(0, HS, 2, dtype=np.float64) / HS))  # [32]
    t = np.arange(T, dtype=np.float64)
    fr = t[:, None] * inv_freq[None, :]          # [T, 32]
    emb = np.concatenate([fr, fr], axis=1)       # [T, 64]
    cos = np.cos(emb).T.astype(np.float32)       # [64, T]
    sin = np.sin(emb).T.astype(np.float32)       # [64, T]
    sin_signed = sin.copy()
    sin_signed[0:32] = -sin_signed[0:32]
    cos2 = np.concatenate([cos, cos], axis=0)            # [128, T]
    sin2 = np.concatenate([sin_signed, sin_signed], 0)   # [128, T]
    return cos2.astype(ml_dtypes.bfloat16), sin2.astype(ml_dtypes.bfloat16)


def _in_maps(x, attention_mask, Wq, bqv, Wk, bkv, Wv, bvv, Wo):
    cos2, sin2 = _rope_tables()
    maskt = (NEGM * np.triu(np.ones((128, 128), np.float32), k=1)
             ).astype(ml_dtypes.bfloat16)
    id2 = np.concatenate([np.eye(128, dtype=np.float32)] * 2, axis=1
                         ).astype(ml_dtypes.bfloat16)
    bf = ml_dtypes.bfloat16
    xTs = [np.ascontiguousarray(x[b].T).astype(bf) for b in range(B)]
    kbs = [
        np.where(attention_mask[b] != 0, 0.0, NEG).astype(np.float32)
        for b in range(B)
    ]
    maps = []
    for core in range(NCORES):
        b, g = core // 4, core % 4
        sl = slice(g * HD, (g + 1) * HD)
        # bias columns [128, 4]: [:, 2*ti+c2]
        bcol = np.stack([
            bqv[sl][0:128], bqv[sl][128:256],
            bkv[sl][0:128], bkv[sl][128:256],
        ], axis=1).astype(np.float32)
        maps.append({
            "xT": xTs[b],
            "wq": np.ascontiguousarray(Wq[:, sl]).astype(bf),
            "wk": np.ascontiguousarray(Wk[:, sl]).astype(bf),
            "wv": np.ascontiguousarray(Wv[:, sl]).astype(bf),
            "wo": np.ascontiguousarray(Wo[sl, :]).astype(bf),
            "bcol": bcol,
            "bvrow": bvv[sl].astype(bf).reshape(1, HD),
            "cos2": cos2,
            "sin2": sin2,
            "maskt": maskt,
            "id2": id2,
            "kb": kbs[b],
        })
    return maps


def _run(inputs, trace=False):
    global _NC
    if _NC is None:
        _NC = build_nc()
    maps = _in_maps(
        np.asarray(inputs["x"]), np.asarray(inputs["attention_mask"]),
        np.asarray(inputs["Wq"]), np.asarray(inputs["bq"]),
        np.asarray(inputs["Wk"]), np.asarray(inputs["bk"]),
        np.asarray(inputs["Wv"]), np.asarray(inputs["bv"]),
        np.asarray(inputs["Wo"]),
    )
    res = run_bass_kernel_spmd(_NC, maps, core_ids=list(range(NCORES)), trace=trace)
    bo = np.asarray(inputs["bo"], np.float32)
    outs = []
    for b in range(B):
        acc = np.zeros((T, D), np.float32)
        for g in range(4):
            acc += np.asarray(res.results[b * 4 + g]["out"], np.float32)
        outs.append(acc + bo[None, :])
    return np.stack(outs, axis=0), res


def kernel(**inputs):
    out, _ = _run(inputs, trace=False)
    return out
